# revision 1
# baseline (speedup 1.0000x reference)
"""Bass/Trainium2 kernel for nn_BerpXposMultiHeadedAttention (8-core SPMD).

Sharding: data-parallel over batch (4 batches x 2 cores) x tensor-parallel over
heads (4 heads per core).  Each core computes its 4 heads of flash-style xpos
attention for its batch plus the row-sharded partial out-projection; the host
sums the two partials per batch (the "all-reduce") and adds the output bias.

Measured-rate design notes (TRN2, via microbenchmarks):
- PE weight loads for 4-byte dtypes are very expensive (~1.4-5.6us exposed per
  [128,128] fp32/fp32r LDWEIGHTS) while bf16 weight loads hide under streams.
  => every frequently-changing stationary operand is bf16.
- Mixing 32-bit and 16-bit matmul inputs is illegal; bf16 x fp16 is legal and
  exact, so projections use bf16 weights (stationary) with fp16 activations
  (moving).  QK^T keeps fp32r x fp32r (K=64 fp32r LDW measured cheap) for
  score accuracy in front of exp.
- xpos rotation: the rotate-half partner comes from a second projection whose
  weight rows are permuted on the host (sin-projection), so no on-device
  cross-partition ops are needed.  cos/sin projections share one [128,1024]
  PSUM tile.
- P@V streams 512 t-columns per matmul with the (v | ones-col) tile as a
  128-wide bf16 stationary; PSUM row 64 is the softmax denominator.  The
  normalization divides 64 rows by a DMA-broadcast denominator row.
- Emission interleaves projection strips with flash strips so all engines ramp
  within ~15us; the causal fast path skips above-diagonal blocks entirely.
"""

import sys

sys.path.insert(0, "/opt/trn_rl_repo")

import contextlib

import numpy as np

import concourse.bacc as bacc
import concourse.bass as bass
import concourse.tile as tile
from concourse import mybir
from concourse.bass_utils import run_bass_kernel_spmd

# Problem constants (hardcoded per the task contract).
B = 4
L = 2048
EMBED = 512
HEADS = 8
HD = 64
SCALE_BASE = 512
NEG = -1e9
N_CORES = 8
HPC = 4           # heads per core
TB = 512          # t-block (strip) width
NT = L // 128     # 16 t-chunks
NS = L // 128     # 16 s-chunks
NSTRIP = L // TB  # 4 strips
VW = 328          # v_aug tile width (4 heads x 65 + 68 pad)

F32 = mybir.dt.float32
F32R = mybir.dt.float32r
F16 = mybir.dt.float16
BF16 = mybir.dt.bfloat16

# Deinterleave permutation of a 64-wide head dim: evens then odds.
_PERM64 = np.concatenate([np.arange(0, HD, 2), np.arange(1, HD, 2)])


def _xpos_tables():
    """Host-side xpos cos/sin tables in the permuted [d, t] layout.

    Returns (cq, sq, ck, sk), each [128, L] float32 (two heads' worth of rows,
    identical per head).  The 1/sqrt(HD) score scale is folded into the q pair.
    """
    d = HD
    base = ((np.arange(0, d, 2, dtype=np.float32) + np.float32(0.4 * d))
            / np.float32(1.4 * d)).astype(np.float32)                    # [32]
    min_pos = -(L // 2)
    power = (np.arange(min_pos, L + min_pos, dtype=np.float32)
             / np.float32(SCALE_BASE))                                   # [L]
    scale = (base[None, :] ** power[:, None]).astype(np.float32)         # [L, 32]
    half = d // 2
    inv_freq = (1.0 / (10000.0 ** (np.arange(half, dtype=np.float32) / half))
                ).astype(np.float32)
    sinusoid = np.arange(L, dtype=np.float32)[:, None] * inv_freq[None, :]
    sin = np.sin(sinusoid).astype(np.float32)
    cos = np.cos(sinusoid).astype(np.float32)

    def pack(cs, ss, fold):
        cs = (cs * fold).astype(np.float32)
        ss = (ss * fold).astype(np.float32)
        # permuted layout: rows 0:32 <- even orig dims, rows 32:64 <- odd.
        cos_p = np.concatenate([cs.T, cs.T], axis=0)      # [64, L]
        sin_p = np.concatenate([-ss.T, ss.T], axis=0)     # [64, L]
        return (np.concatenate([cos_p, cos_p], axis=0).astype(np.float32),
                np.concatenate([sin_p, sin_p], axis=0).astype(np.float32))

    inv_scale = (1.0 / scale).astype(np.float32)
    cq, sq = pack(cos * scale, sin * scale, np.float32(HD ** -0.5))
    ck, sk = pack(cos * inv_scale, sin * inv_scale, np.float32(1.0))
    return cq, sq, ck, sk


def _build_program(causal: bool, use_mask: bool, has_bias: bool, reps: int = 1, debug_taps: bool = False):
    nc = bacc.Bacc("TRN2", target_bir_lowering=False, debug=False,
                   num_devices=N_CORES)

    # ---- DRAM I/O -------------------------------------------------------
    xqT = nc.dram_tensor("xqT", [513, L], F16, kind="ExternalInput")
    xkT = nc.dram_tensor("xkT", [513, L], F16, kind="ExternalInput")
    xvT = nc.dram_tensor("xvT", [513, L], F16, kind="ExternalInput")
    wqcT = nc.dram_tensor("wqcT", [513, 256], BF16, kind="ExternalInput")
    wqsT = nc.dram_tensor("wqsT", [513, 256], BF16, kind="ExternalInput")
    wkcT = nc.dram_tensor("wkcT", [513, 256], BF16, kind="ExternalInput")
    wksT = nc.dram_tensor("wksT", [513, 256], BF16, kind="ExternalInput")
    wvT = nc.dram_tensor("wvT", [513, 256], BF16, kind="ExternalInput")
    woT = nc.dram_tensor("woT", [256, EMBED], BF16, kind="ExternalInput")
    cqD = nc.dram_tensor("cq", [128, L], F32, kind="ExternalInput")
    sqD = nc.dram_tensor("sq", [128, L], F32, kind="ExternalInput")
    ckD = nc.dram_tensor("ck", [128, L], F32, kind="ExternalInput")
    skD = nc.dram_tensor("sk", [128, L], F32, kind="ExternalInput")
    triD = nc.dram_tensor("tri", [128, 128], F32, kind="ExternalInput")
    maskD = None
    if use_mask:
        maskD = nc.dram_tensor("maskT", [L, L], F32, kind="ExternalInput")
    outp = nc.dram_tensor("outp", [L, EMBED], F32, kind="ExternalOutput")
    dbg = {}
    if debug_taps:
        dbg["qT00"] = nc.dram_tensor("dbg_qT00", [128, TB], F32, kind="ExternalOutput")
        dbg["kT00"] = nc.dram_tensor("dbg_kT00", [128, TB], F32, kind="ExternalOutput")
        dbg["vaug0"] = nc.dram_tensor("dbg_vaug0", [128, VW], F32, kind="ExternalOutput")
        dbg["pt000"] = nc.dram_tensor("dbg_pt000", [128, 1024], F32, kind="ExternalOutput")
        dbg["po00"] = nc.dram_tensor("dbg_po00", [128, TB], F32, kind="ExternalOutput")
        dbg["sums00"] = nc.dram_tensor("dbg_sums00", [64, TB], F32, kind="ExternalOutput")
        dbg["attnT0"] = nc.dram_tensor("dbg_attnT0", [128, L], F32, kind="ExternalOutput")

    xin = {"q": xqT, "k": xkT, "v": xvT}
    win = {"qc": wqcT, "qs": wqsT, "kc": wkcT, "ks": wksT, "v": wvT}
    tabin = {"cq": cqD, "sq": sqD, "ck": ckD, "sk": skD}

    with tile.TileContext(nc) as tc:
        with contextlib.ExitStack() as ctx:
            consts = ctx.enter_context(tc.tile_pool(name="consts", bufs=1))
            xpool = ctx.enter_context(tc.tile_pool(name="xpool", bufs=14))
            wpool = ctx.enter_context(tc.tile_pool(name="wpool", bufs=1))
            qkpool = ctx.enter_context(tc.tile_pool(name="qkpool", bufs=1))
            tabpool = ctx.enter_context(tc.tile_pool(name="tabpool", bufs=6))
            vpool = ctx.enter_context(tc.tile_pool(name="vpool", bufs=NS))
            tmp = ctx.enter_context(tc.tile_pool(name="tmp", bufs=3))
            ptpool = ctx.enter_context(tc.tile_pool(name="ptpool", bufs=10))
            npool = ctx.enter_context(tc.tile_pool(name="npool", bufs=3))
            opool = ctx.enter_context(tc.tile_pool(name="opool", bufs=2))
            drpool = ctx.enter_context(
                tc.tile_pool(name="drpool", bufs=3, space="DRAM"))
            mpool = None
            if use_mask:
                mpool = ctx.enter_context(tc.tile_pool(name="mpool", bufs=NS + 2))
            ps_s = ctx.enter_context(tc.tile_pool(name="ps_s", bufs=3, space="PSUM"))
            ps_pv = ctx.enter_context(tc.tile_pool(name="ps_pv", bufs=2, space="PSUM"))

            def body():
                # ---- stage-0 small constants ----
                ones_sb = consts.tile([1, L], F16, tag="ones")
                nc.sync.dma_start(ones_sb[:], xqT[512:513, :])
                tri_sb = consts.tile([128, 128], F32, tag="tri")
                if causal:
                    nc.sync.dma_start(tri_sb[:], triD[:])

                w_sb = {}

                def load_w(nm):
                    chunks = []
                    for c in range(4):
                        t = wpool.tile([128, 256], BF16, tag=f"w{nm}{c}",
                                       name=f"w{nm}{c}")
                        nc.sync.dma_start(t[:], win[nm][c * 128:(c + 1) * 128, :])
                        chunks.append(t)
                    bt = None
                    if has_bias:
                        bt = wpool.tile([1, 256], BF16, tag=f"w{nm}b",
                                        name=f"w{nm}b")
                        nc.sync.dma_start(bt[:], win[nm][512:513, :])
                    w_sb[nm] = (chunks, bt)

                load_w("qc")
                load_w("qs")

                attnT = [consts.tile([128, L], BF16, tag=f"attnT{c}",
                                     name=f"attnT{c}") for c in range(2)]

                qTt = [[None] * NSTRIP for _ in range(2)]  # [e][tb]
                kTt = [[None] * NSTRIP for _ in range(2)]
                vaug = [None] * NS

                def load_x_tb(nm, tb):
                    pieces = []
                    for c in range(4):
                        t = xpool.tile([128, TB], F16, tag="x",
                                       name=f"x{nm}{c}_{tb}")
                        nc.sync.dma_start(
                            t[:], xin[nm][c * 128:(c + 1) * 128,
                                          tb * TB:(tb + 1) * TB])
                        pieces.append(t)
                    return pieces

                def load_tab(nm, tb):
                    t = tabpool.tile([128, TB], F32, tag=nm, name=f"{nm}{tb}")
                    nc.sync.dma_start(
                        t[:], tabin[nm][:, tb * TB:(tb + 1) * TB])
                    return t

                def proj_qk(nm, tb, xs, ctab, stab, dst):
                    wc, wcb = w_sb[nm + "c"]
                    ws, wsb = w_sb[nm + "s"]
                    on = ones_sb[:, tb * TB:(tb + 1) * TB]
                    for e in range(2):
                        es = slice(e * 128, (e + 1) * 128)
                        ps = ps_s.tile([128, 1024], F32, tag="s",
                                       name=f"ps_{nm}{e}_{tb}")
                        for c in range(4):
                            nc.tensor.matmul(ps[:, 0:TB], wc[c][:, es], xs[c][:],
                                             start=(c == 0),
                                             stop=(c == 3 and not has_bias))
                        if has_bias:
                            nc.tensor.matmul(ps[:, 0:TB], wcb[:, es], on,
                                             start=False, stop=True)
                        for c in range(4):
                            nc.tensor.matmul(ps[:, TB:1024], ws[c][:, es], xs[c][:],
                                             start=(c == 0),
                                             stop=(c == 3 and not has_bias))
                        if has_bias:
                            nc.tensor.matmul(ps[:, TB:1024], wsb[:, es], on,
                                             start=False, stop=True)
                        t1 = tmp.tile([128, TB], F32, tag="t1",
                                      name=f"t1{nm}{e}{tb}")
                        nc.vector.tensor_mul(t1[:], ps[:, 0:TB], ctab[:])
                        t2 = tmp.tile([128, TB], F32, tag="t2",
                                      name=f"t2{nm}{e}{tb}")
                        nc.vector.tensor_mul(t2[:], ps[:, TB:1024], stab[:])
                        ot = qkpool.tile([128, TB], F32R, tag=f"{nm}T{e}_{tb}",
                                         name=f"{nm}T{e}_{tb}")
                        nc.vector.tensor_add(ot[:], t1[:], t2[:])
                        dst[e][tb] = ot
                        if debug_taps and tb == 0 and e == 0:
                            dt_ = tmp.tile([128, TB], F32, tag="dbgc",
                                           name=f"dbg{nm}", bufs=1)
                            nc.vector.tensor_copy(dt_[:], ot[:])
                            nc.sync.dma_start(
                                dbg["qT00" if nm == "q" else "kT00"][:], dt_[:])

                def proj_v(tb, xs):
                    wv, wvb = w_sb["v"]
                    for j in range(4):
                        si = tb * 4 + j
                        js = slice(j * 128, (j + 1) * 128)
                        ps = ps_pv.tile([128, 256], F32, tag="pv",
                                        name=f"ps_v{si}")
                        for c in range(4):
                            nc.tensor.matmul(ps[:], xs[c][:, js], wv[c][:],
                                             start=(c == 0),
                                             stop=(c == 3 and not has_bias))
                        if has_bias:
                            nc.tensor.matmul(
                                ps[:], ones_sb[:, si * 128:(si + 1) * 128],
                                wvb[:], start=False, stop=True)
                        va = vpool.tile([128, VW], BF16, tag="vaug",
                                        name=f"vaug{si}")
                        va3 = va[:, 0:HPC * 65].rearrange("p (h c) -> p h c", c=65)
                        nc.vector.tensor_copy(
                            va3[:, :, 0:64],
                            ps[:].rearrange("p (h d) -> p h d", d=64))
                        nc.vector.memset(va3[:, :, 64:65], 1.0)
                        nc.vector.memset(va[:, HPC * 65:VW], 0.0)
                        vaug[si] = va
                        if debug_taps and si == 0:
                            dt_ = tmp.tile([128, VW], F32, tag="dbgv",
                                           name="dbgv", bufs=1)
                            nc.vector.tensor_copy(dt_[:], va[:])
                            nc.sync.dma_start(dbg["vaug0"][:], dt_[:])

                def flash_strip(T):
                    nsig = 4 * T + 4 if causal else NS
                    mtiles = None
                    if use_mask:
                        mtiles = []
                        for si in range(nsig):
                            mt = mpool.tile([128, TB], F32, tag="mask",
                                            name=f"m{T}_{si}")
                            nc.sync.dma_start(
                                mt[:], maskD[si * 128:(si + 1) * 128,
                                             T * TB:(T + 1) * TB])
                            mtiles.append(mt)
                    for h in range(HPC):
                        ht, hr = h // 2, (h % 2) * 64
                        pts = []
                        for g in range((nsig + 1) // 2):
                            ps2 = ps_s.tile([128, 1024], F32, tag="s",
                                            name=f"S{T}h{h}g{g}")
                            for u in range(2):
                                sig = g * 2 + u
                                if sig >= nsig:
                                    continue
                                j = sig - 4 * T
                                coff = 0
                                if causal and j >= 0:
                                    ncols = max(TB - j * 128, 256)
                                    coff = TB - ncols
                                nc.tensor.matmul(
                                    ps2[:, u * TB + coff:(u + 1) * TB],
                                    kTt[ht][sig // 4][hr:hr + 64,
                                                      (sig % 4) * 128:
                                                      (sig % 4 + 1) * 128],
                                    qTt[ht][T][hr:hr + 64, coff:TB],
                                    start=True, stop=True)
                                if causal and j >= 0:
                                    sl = slice(u * TB + j * 128,
                                               u * TB + (j + 1) * 128)
                                    nc.vector.tensor_add(ps2[:, sl], ps2[:, sl],
                                                         tri_sb[:])
                                if use_mask:
                                    sl = slice(u * TB, (u + 1) * TB)
                                    nc.vector.tensor_add(ps2[:, sl], ps2[:, sl],
                                                         mtiles[sig][:])
                            pt = ptpool.tile([128, 1024], BF16, tag="pt",
                                             name=f"P{T}h{h}g{g}")
                            nc.scalar.activation(pt[:], ps2[:],
                                                 mybir.ActivationFunctionType.Exp)
                            if causal:
                                for u in range(2):
                                    j = g * 2 + u - 4 * T
                                    if 1 <= j <= 3:
                                        nc.vector.memset(
                                            pt[:, u * TB:u * TB + j * 128], 0.0)
                            pts.append(pt)
                            if debug_taps and T == 0 and h == 0 and g == 0:
                                dt_ = tmp.tile([128, 1024], F32, tag="dbgp",
                                               name="dbgp", bufs=1)
                                nc.vector.tensor_copy(dt_[:], pt[:])
                                nc.sync.dma_start(dbg["pt000"][:], dt_[:])
                        # P@V: psum rows 0:64 attnU.T, row 64 sumexp, rest junk
                        po = ps_pv.tile([128, TB], F32, tag="pv",
                                        name=f"po{T}h{h}")
                        for sig in range(nsig):
                            nc.tensor.matmul(
                                po[:], vaug[sig][:, h * 65:h * 65 + 128],
                                pts[sig // 2][:, (sig % 2) * TB:
                                              (sig % 2 + 1) * TB],
                                start=(sig == 0), stop=(sig == nsig - 1))
                        # normalization part 1: free the psum, launch the
                        # denominator broadcast; recip+divide are deferred to
                        # the next strip so the DMA latency never stalls DVE.
                        if debug_taps and T == 0 and h == 0:
                            dt_ = npool.tile([128, TB], F32, tag="dbgo",
                                             name="dbgo", bufs=1)
                            nc.vector.tensor_copy(dt_[:], po[:])
                            nc.sync.dma_start(dbg["po00"][:], dt_[:])
                        poc = npool.tile([65, TB], F32, tag="poc",
                                         name=f"poc{T}h{h}", bufs=8)
                        nc.vector.tensor_copy(poc[:], po[0:65, :])
                        dbn = drpool.tile([1, TB], F32, tag="dbn",
                                          name=f"dbn{T}h{h}")
                        nc.sync.dma_start(dbn[:], poc[64:65, :])
                        sums = npool.tile([64, TB], F32, tag="sums",
                                          name=f"sums{T}h{h}", bufs=8)
                        bcast = bass.AP(tensor=dbn[:].tensor, offset=dbn[:].offset,
                                        ap=[[0, 64], [1, TB]])
                        nc.sync.dma_start(sums[:], bcast)
                        pending_norm.append((poc, sums, ht, hr, T))
                        if debug_taps and T == 0 and h == 0:
                            nc.sync.dma_start(dbg["sums00"][:], sums[:])

                def drain_norms():
                    while pending_norm:
                        poc, sums, ht, hr, T = pending_norm.pop(0)
                        rec = npool.tile([64, TB], F32, tag="rec",
                                         name=f"rec{T}x{ht}{hr}")
                        nc.vector.reciprocal(rec[:], sums[:])
                        tcols = slice(T * TB, (T + 1) * TB)
                        if hr == 0:
                            nc.vector.tensor_mul(attnT[ht][0:64, tcols],
                                                 poc[0:64, :], rec[:])
                        else:
                            stag = npool.tile([64, TB], BF16, tag="stag",
                                              name=f"stag{T}x{ht}{hr}")
                            nc.vector.tensor_mul(stag[:], poc[0:64, :], rec[:])
                            nc.sync.dma_start(attnT[ht][64:128, tcols], stag[:])

                def out_proj():
                    for tau in range(NT):
                        ps = ps_s.tile([128, EMBED], F32, tag="s",
                                       name=f"ps_o{tau}")
                        for c in range(2):
                            nc.tensor.matmul(
                                ps[:], attnT[c][:, tau * 128:(tau + 1) * 128],
                                wo_sb[c][:], start=(c == 0), stop=(c == 1))
                        osb = opool.tile([128, EMBED], F32, tag="osb",
                                         name=f"osb{tau}")
                        nc.any.tensor_copy(osb[:], ps[:])
                        nc.sync.dma_start(outp[tau * 128:(tau + 1) * 128, :],
                                          osb[:])

                wo_sb = None
                pending_norm = []
                for tb in range(NSTRIP):
                    xs = load_x_tb("q", tb)
                    ct, st = load_tab("cq", tb), load_tab("sq", tb)
                    proj_qk("q", tb, xs, ct, st, qTt)
                    if tb == 0:
                        load_w("kc")
                        load_w("ks")
                    xs = load_x_tb("k", tb)
                    ct, st = load_tab("ck", tb), load_tab("sk", tb)
                    proj_qk("k", tb, xs, ct, st, kTt)
                    if tb == 0:
                        load_w("v")
                    xs = load_x_tb("v", tb)
                    proj_v(tb, xs)
                    if tb == 0:
                        wo_sb = []
                        for c in range(2):
                            t = consts.tile([128, EMBED], BF16, tag=f"wo{c}",
                                            name=f"wo{c}")
                            nc.sync.dma_start(t[:], woT[c * 128:(c + 1) * 128, :])
                            wo_sb.append(t)
                    norms_due = pending_norm[:]
                    flash_strip(tb)
                    if tb > 0 or not causal:
                        del pending_norm[:len(norms_due)]
                        pending_norm_tail = pending_norm[:]
                        pending_norm.clear()
                        pending_norm.extend(norms_due)
                        drain_norms()
                        pending_norm.extend(pending_norm_tail)
                if debug_taps:
                    dt_ = consts.tile([128, L], F32, tag="dbga", name="dbga")
                    nc.vector.tensor_copy(dt_[:], attnT[0][:])
                    nc.sync.dma_start(dbg["attnT0"][:], dt_[:])
                drain_norms()
                out_proj()

            if reps > 1:
                with tc.For_i(0, reps, 1,
                              hint_engines=(mybir.EngineType.PE,
                                            mybir.EngineType.Activation,
                                            mybir.EngineType.DVE,
                                            mybir.EngineType.SP,
                                            mybir.EngineType.Pool)):
                    body()
            else:
                body()

    nc.compile()
    return nc


_PROGRAM_CACHE = {}


def get_program(causal: bool, use_mask: bool, has_bias: bool, reps: int = 1):
    key = (causal, use_mask, has_bias, reps)
    if key not in _PROGRAM_CACHE:
        _PROGRAM_CACHE[key] = _build_program(causal, use_mask, has_bias, reps)
    return _PROGRAM_CACHE[key]


def _prep_in_maps(query, key, value, key_padding_mask, attn_mask,
                  Wq, bq, Wk, bk, Wv, bv, Wo, bo, use_mask, has_bias):
    """Build the 8 per-core input dicts."""
    import ml_dtypes
    cq, sq, ck, sk = _xpos_tables()
    tri = np.where(np.arange(128)[None, :] >= np.arange(128)[:, None],
                   np.float32(0.0), np.float32(NEG)).astype(np.float32)

    def aug_x(x):
        a = np.empty((513, L), np.float16)
        a[0:512] = np.asarray(x, np.float32).T.astype(np.float16)
        a[512] = np.float16(1.0)
        return a

    xqTs = [aug_x(query[b]) for b in range(B)]
    xkTs = [aug_x(key[b]) for b in range(B)]
    xvTs = [aug_x(value[b]) for b in range(B)]

    masks = None
    if use_mask:
        am = np.asarray(attn_mask, np.float32)
        kp = np.asarray(key_padding_mask)
        masks = []
        for b in range(B):
            m = am.copy()
            if kp[b].any():
                m = m + np.where(kp[b], np.float32(-1e30),
                                 np.float32(0.0))[None, :]
            masks.append(np.ascontiguousarray(m.T.astype(np.float32)))

    Wq = np.asarray(Wq, np.float32); bq = np.asarray(bq, np.float32)
    Wk = np.asarray(Wk, np.float32); bk = np.asarray(bk, np.float32)
    Wv = np.asarray(Wv, np.float32); bv = np.asarray(bv, np.float32)
    Wo = np.asarray(Wo, np.float32)

    in_maps = []
    for core in range(N_CORES):
        b, hg = core // 2, core % 2
        hs = hg * HPC
        idx_p = np.concatenate(
            [hs * HD + hl * HD + _PERM64 for hl in range(HPC)])
        # sin-projection rows: within each head's 64-block, row r <- r XOR 32
        xor = (np.arange(256).reshape(HPC, HD)[:, (np.arange(HD) ^ 32)]
               ).reshape(-1)
        idx_s = idx_p[xor]
        idx_v = hs * HD + np.arange(HPC * HD)

        def aug_w(W, bias, idx):
            a = np.empty((513, 256), np.float32)
            a[0:512] = np.ascontiguousarray(W[idx, :].T)
            a[512] = bias[idx]
            return a.astype(ml_dtypes.bfloat16)

        m = {
            "xqT": xqTs[b], "xkT": xkTs[b], "xvT": xvTs[b],
            "wqcT": aug_w(Wq, bq, idx_p),
            "wqsT": aug_w(Wq, bq, idx_s),
            "wkcT": aug_w(Wk, bk, idx_p),
            "wksT": aug_w(Wk, bk, idx_s),
            "wvT": aug_w(Wv, bv, idx_v),
            "woT": np.ascontiguousarray(Wo[:, idx_v].T).astype(ml_dtypes.bfloat16),
            "cq": cq, "sq": sq, "ck": ck, "sk": sk,
            "tri": tri,
        }
        if use_mask:
            m["maskT"] = masks[b]
        in_maps.append(m)
    return in_maps


def classify_mask(attn_mask, key_padding_mask):
    am = np.asarray(attn_mask, np.float32)
    kp = np.asarray(key_padding_mask)
    if not kp.any():
        causal = np.where(
            np.tril(np.ones((L, L), bool)), np.float32(0.0),
            np.float32(NEG)).astype(np.float32)
        if np.array_equal(am, causal):
            return True, False
        if not am.any():
            return False, False
    return False, True


def kernel(query, key, value, key_padding_mask, attn_mask,
           Wq, bq, Wk, bk, Wv, bv, Wo, bo):
    causal, use_mask = classify_mask(attn_mask, key_padding_mask)
    has_bias = bool(np.asarray(bq).any() or np.asarray(bk).any()
                    or np.asarray(bv).any())
    nc = get_program(causal, use_mask, has_bias, reps=1)
    in_maps = _prep_in_maps(query, key, value, key_padding_mask, attn_mask,
                            Wq, bq, Wk, bk, Wv, bv, Wo, bo, use_mask, has_bias)
    res = run_bass_kernel_spmd(nc, in_maps, list(range(N_CORES)))
    bo = np.asarray(bo, np.float32)
    out = np.empty((B, L, EMBED), np.float32)
    for b in range(B):
        out[b] = (res.results[2 * b]["outp"]
                  + res.results[2 * b + 1]["outp"] + bo[None, :])
    return out



# revision 32
# speedup vs baseline: 2.4012x; 2.4012x over previous
"""Bass/Trainium2 kernel for nn_BerpXposMultiHeadedAttention (8-core SPMD).

Sharding: data-parallel over batch (4 batches x 2 cores) x tensor-parallel over
heads (4 heads per core).  Each core computes its 4 heads of flash-style xpos
attention for its batch plus the row-sharded partial out-projection; the host
sums the two partials per batch (the "all-reduce") and adds the output bias.

Design notes (v2, rebalanced from HW slope measurements + TimelineSim):
- All matmul operands are 16-bit: bf16 weights (stationary) x fp16 activations,
  fp16 q/k for QK^T (FWL weight loads), bf16 probabilities for P@V.  fp32
  PSUM accumulation throughout keeps the softmax inputs accurate.
- xpos rotation via dual projection (cos-path and sin-path with host-permuted
  weight rows); the elementwise combine runs as two DVE muls (fp16 tables) and
  one gpsimd (Pool) add, keeping DVE off the critical path.
- Causal fast path trims score matmuls, exp consumption, and P@V moving
  columns to the exact 128-block diagonal; only the diagonal 128x128 block
  needs a triangle mask (DVE add of a -1e9 upper-triangle on PSUM).
- Softmax normalization: P@V accumulates an extra ones-row per head (PSUM row
  64 = sumexp).  reciprocal_approx_fast on the [1,512] denominator row, DRAM
  round-trip broadcast to [64,512], multiply on the Pool engine.  No 6-cpe
  DVE reciprocal, no [64,512] DVE multiply.
- Emission interleaves projection strips with flash strips so all engines ramp
  early; the causal fast path skips above-diagonal blocks entirely.
"""

import sys

sys.path.insert(0, "/opt/trn_rl_repo")

import contextlib

import numpy as np

import concourse.bacc as bacc
import concourse.bass as bass
import concourse.tile as tile
from concourse import mybir
from concourse.bass_utils import run_bass_kernel_spmd

# Problem constants (hardcoded per the task contract).
B = 4
L = 2048
EMBED = 512
HEADS = 8
HD = 64
SCALE_BASE = 512
NEG = -1e9
N_CORES = 8
HPC = 4           # heads per core
TB = 512          # t-block (strip) width
NT = L // 128     # 16 t-chunks
NS = L // 128     # 16 s-chunks
NSTRIP = L // TB  # 4 strips
VW = 328          # v_aug tile width (4 heads x 65 + 68 junk tail)

F32 = mybir.dt.float32
F32R = mybir.dt.float32r
F16 = mybir.dt.float16
BF16 = mybir.dt.bfloat16

# Deinterleave permutation of a 64-wide head dim: evens then odds.
_PERM64 = np.concatenate([np.arange(0, HD, 2), np.arange(1, HD, 2)])


def _xpos_tables():
    """Host-side xpos cos/sin tables in the permuted [d, t] layout.

    Returns (cq, sq, ck, sk), each [128, L] float16 (two heads' worth of rows,
    identical per head).  The 1/sqrt(HD) score scale is folded into the q pair.
    """
    d = HD
    base = ((np.arange(0, d, 2, dtype=np.float32) + np.float32(0.4 * d))
            / np.float32(1.4 * d)).astype(np.float32)                    # [32]
    min_pos = -(L // 2)
    power = (np.arange(min_pos, L + min_pos, dtype=np.float32)
             / np.float32(SCALE_BASE))                                   # [L]
    scale = (base[None, :] ** power[:, None]).astype(np.float32)         # [L, 32]
    half = d // 2
    inv_freq = (1.0 / (10000.0 ** (np.arange(half, dtype=np.float32) / half))
                ).astype(np.float32)
    sinusoid = np.arange(L, dtype=np.float32)[:, None] * inv_freq[None, :]
    sin = np.sin(sinusoid).astype(np.float32)
    cos = np.cos(sinusoid).astype(np.float32)

    def pack(cs, ss, fold):
        cs = (cs * fold).astype(np.float32)
        ss = (ss * fold).astype(np.float32)
        # permuted layout: rows 0:32 <- even orig dims, rows 32:64 <- odd.
        cos_p = np.concatenate([cs.T, cs.T], axis=0)      # [64, L]
        sin_p = np.concatenate([-ss.T, ss.T], axis=0)     # [64, L]
        return (np.concatenate([cos_p, cos_p], axis=0).astype(np.float16),
                np.concatenate([sin_p, sin_p], axis=0).astype(np.float16))

    inv_scale = (1.0 / scale).astype(np.float32)
    cq, sq = pack(cos * scale, sin * scale, np.float32(HD ** -0.5))
    ck, sk = pack(cos * inv_scale, sin * inv_scale, np.float32(1.0))
    return cq, sq, ck, sk


def _build_program(causal: bool, use_mask: bool, has_bias: bool, reps: int = 1):
    nc = bacc.Bacc("TRN2", target_bir_lowering=False, debug=False,
                   num_devices=N_CORES)

    # ---- DRAM I/O -------------------------------------------------------
    xqT = nc.dram_tensor("xqT", [513, L], F16, kind="ExternalInput")
    xkT = nc.dram_tensor("xkT", [513, L], F16, kind="ExternalInput")
    xvT = nc.dram_tensor("xvT", [513, L], F16, kind="ExternalInput")
    wqcT = nc.dram_tensor("wqcT", [513, 256], BF16, kind="ExternalInput")
    wqsT = nc.dram_tensor("wqsT", [513, 256], BF16, kind="ExternalInput")
    wkcT = nc.dram_tensor("wkcT", [513, 256], BF16, kind="ExternalInput")
    wksT = nc.dram_tensor("wksT", [513, 256], BF16, kind="ExternalInput")
    wvT = nc.dram_tensor("wvT", [513, 256], BF16, kind="ExternalInput")
    woT = nc.dram_tensor("woT", [256, EMBED], BF16, kind="ExternalInput")
    cqD = nc.dram_tensor("cq", [128, L], F16, kind="ExternalInput")
    sqD = nc.dram_tensor("sq", [128, L], F16, kind="ExternalInput")
    ckD = nc.dram_tensor("ck", [128, L], F16, kind="ExternalInput")
    skD = nc.dram_tensor("sk", [128, L], F16, kind="ExternalInput")
    triD = nc.dram_tensor("tri", [128, 128], F16, kind="ExternalInput")
    maskD = None
    if use_mask:
        maskD = nc.dram_tensor("maskT", [L, L], F32, kind="ExternalInput")
    outp = nc.dram_tensor("outp", [L, EMBED], F16, kind="ExternalOutput")

    xin = {"q": xqT, "k": xkT, "v": xvT}
    win = {"qc": wqcT, "qs": wqsT, "kc": wkcT, "ks": wksT, "v": wvT}
    tabin = {"cq": cqD, "sq": sqD, "ck": ckD, "sk": skD}

    with tile.TileContext(nc) as tc:
        with contextlib.ExitStack() as ctx:
            consts = ctx.enter_context(tc.tile_pool(name="consts", bufs=1))
            xpool = ctx.enter_context(tc.tile_pool(name="xpool", bufs=1))
            wpool = ctx.enter_context(tc.tile_pool(name="wpool", bufs=1))
            qkpool = ctx.enter_context(tc.tile_pool(name="qkpool", bufs=1))
            tabpool = ctx.enter_context(tc.tile_pool(name="tabpool", bufs=1))
            vpool = ctx.enter_context(tc.tile_pool(name="vpool", bufs=NS + 4))
            tmp = ctx.enter_context(tc.tile_pool(name="tmp", bufs=3))
            ptpool = ctx.enter_context(tc.tile_pool(name="ptpool", bufs=10))
            npool = ctx.enter_context(tc.tile_pool(name="npool", bufs=3))
            opool = ctx.enter_context(tc.tile_pool(name="opool", bufs=2))
            drpool = ctx.enter_context(
                tc.tile_pool(name="drpool", bufs=3, space="DRAM"))
            mpool = None
            if use_mask:
                mpool = ctx.enter_context(tc.tile_pool(name="mpool", bufs=NS + 2))
            ps_s = ctx.enter_context(tc.tile_pool(name="ps_s", bufs=2, space="PSUM"))
            ps_sm = ctx.enter_context(tc.tile_pool(name="ps_sm", bufs=2, space="PSUM"))
            ps_pv = ctx.enter_context(tc.tile_pool(name="ps_pv", bufs=2, space="PSUM"))

            def body(prev_tail=None):
                # ---- stage-0 small constants ----
                ones_sb = None
                if has_bias:
                    ones_sb = consts.tile([1, L], F16, tag="ones")
                    nc.sync.dma_start(ones_sb[:], xqT[512:513, :])
                tri_sb = consts.tile([128, 128], F16, tag="tri", bufs=2)
                if causal:
                    nc.sync.dma_start(tri_sb[:], triD[:])

                w_sb = {}

                def load_w(nm):
                    wt = wpool.tile([128, 1024], BF16, tag=f"w{nm}",
                                    name=f"w{nm}")
                    wd = win[nm]
                    nc.sync.dma_start(
                        wt[:], bass.AP(tensor=wd[:].tensor, offset=wd[:].offset,
                                       ap=[[256, 128], [32768, 4], [1, 256]]))
                    chunks = [wt[:, c * 256:(c + 1) * 256] for c in range(4)]
                    bt = None
                    if has_bias:
                        bt = wpool.tile([1, 256], BF16, tag=f"w{nm}b",
                                        name=f"w{nm}b")
                        nc.sync.dma_start(bt[:], win[nm][512:513, :])
                    w_sb[nm] = (chunks, bt)

                attnT = [consts.tile([128, L], F16, tag=f"attnT{c}",
                                     name=f"attnT{c}", bufs=2) for c in range(2)]

                qTt = [[None] * NSTRIP for _ in range(2)]  # [e][tb]
                kTt = [[None] * NSTRIP for _ in range(2)]
                vaug = [None] * NS

                xfull = {}

                def load_x_part(nm, lo, hi):
                    if nm not in xfull:
                        xfull[nm] = [xpool.tile([128, L], F16, tag=f"x{nm}{c}",
                                                name=f"x{nm}{c}")
                                     for c in range(4)]
                    for c in range(4):
                        nc.sync.dma_start(
                            xfull[nm][c][:, lo:hi],
                            xin[nm][c * 128:(c + 1) * 128, lo:hi])

                def load_x_tb(nm, tb):
                    return [t[:, tb * TB:(tb + 1) * TB] for t in xfull[nm]]

                tabfull = {}

                def load_tab_part(nm, lo, hi):
                    if nm not in tabfull:
                        tabfull[nm] = tabpool.tile([128, L], F16, tag=nm,
                                                   name=f"{nm}full")
                    nc.sync.dma_start(tabfull[nm][:, lo:hi],
                                      tabin[nm][:, lo:hi])

                def load_tab(nm, tb):
                    return tabfull[nm][:, tb * TB:(tb + 1) * TB]

                def proj_qk(nm, tb, xs, ctab, stab, dst, es_list=(0, 1)):
                    wc, wcb = w_sb[nm + "c"]
                    ws, wsb = w_sb[nm + "s"]
                    on = (ones_sb[:, tb * TB:(tb + 1) * TB]
                          if has_bias else None)
                    for e in es_list:
                        es = slice(e * 128, (e + 1) * 128)
                        psc = ps_sm.tile([128, TB], F32, tag="sm",
                                         name=f"ps_{nm}c{e}_{tb}")
                        pss = ps_sm.tile([128, TB], F32, tag="sm",
                                         name=f"ps_{nm}s{e}_{tb}")
                        for c in range(4):
                            nc.tensor.matmul(psc[:], wc[c][:, es], xs[c],
                                             start=(c == 0),
                                             stop=(c == 3 and not has_bias))
                        if has_bias:
                            nc.tensor.matmul(psc[:], wcb[:, es], on,
                                             start=False, stop=True)
                        for c in range(4):
                            nc.tensor.matmul(pss[:], ws[c][:, es], xs[c],
                                             start=(c == 0),
                                             stop=(c == 3 and not has_bias))
                        if has_bias:
                            nc.tensor.matmul(pss[:], wsb[:, es], on,
                                             start=False, stop=True)
                        t1 = tmp.tile([128, TB], F32, tag="t1",
                                      name=f"t1{nm}{e}{tb}")
                        nc.vector.tensor_mul(t1[:], psc[:], ctab)
                        t2 = tmp.tile([128, TB], F32, tag="t2",
                                      name=f"t2{nm}{e}{tb}")
                        nc.vector.tensor_mul(t2[:], pss[:], stab)
                        ot = qkpool.tile([128, TB], F16, tag=f"{nm}T{e}_{tb}",
                                         name=f"{nm}T{e}_{tb}")
                        nc.gpsimd.tensor_add(ot[:], t1[:], t2[:])
                        dst[e][tb] = ot

                def proj_v(tb, xs):
                    wv, wvb = w_sb["v"]
                    for j in range(4):
                        si = tb * 4 + j
                        js = slice(j * 128, (j + 1) * 128)
                        ps = ps_sm.tile([128, 256], F32, tag="sm",
                                        name=f"ps_v{si}")
                        for c in range(4):
                            nc.tensor.matmul(ps[:], xs[c][:, js], wv[c][:],
                                             start=(c == 0),
                                             stop=(c == 3 and not has_bias))
                        if has_bias:
                            nc.tensor.matmul(
                                ps[:], ones_sb[:, si * 128:(si + 1) * 128],
                                wvb[:], start=False, stop=True)
                        va = vpool.tile([128, VW], BF16, tag="vaug",
                                        name=f"vaug{si}")
                        va3 = va[:, 0:HPC * 65].rearrange("p (h c) -> p h c", c=65)
                        nc.vector.tensor_copy(
                            va3[:, :, 0:64],
                            ps[:].rearrange("p (h d) -> p h d", d=64))
                        nc.vector.memset(va3[:, :, 64:65], 1.0)
                        vaug[si] = va

                def flash_strip(T, fillers=()):
                    nsig = 4 * T + 4 if causal else NS
                    mtiles = None
                    if use_mask:
                        mtiles = []
                        for si in range(nsig):
                            mt = mpool.tile([128, TB], F32, tag="mask",
                                            name=f"m{T}_{si}")
                            nc.sync.dma_start(
                                mt[:], maskD[si * 128:(si + 1) * 128,
                                             T * TB:(T + 1) * TB])
                            mtiles.append(mt)
                    d4 = drpool.tile([4, TB], F32, tag="d4", name=f"d4{T}")
                    for p in range(2):
                        # heads A=2p (rows 0:64) and B=2p+1 (rows 64:128) of
                        # the pair-tile run as concurrent row-tiled matmuls
                        # into the two halves of a shared [128,1024] tile.
                        po = [ps_pv.tile([128, TB], F32, tag="pv",
                                         name=f"po{T}p{p}h{half}")
                              for half in range(2)]
                        pts = []
                        coffs = {}
                        for sig in range(nsig):
                            j = sig - 4 * T
                            coff = j * 128 if (causal and j > 0) else 0
                            coffs[sig] = coff
                            ps2 = ps_s.tile([128, 1024], F32, tag="s",
                                            name=f"S{T}p{p}s{sig}")
                            kt = kTt[p][sig // 4]
                            qt = qTt[p][T]
                            scols = slice((sig % 4) * 128, (sig % 4 + 1) * 128)
                            for half, hb in ((0, 0), (1, 64)):
                                nc.tensor.matmul(
                                    ps2[:, half * TB + coff:(half + 1) * TB],
                                    kt[hb:hb + 64, scols],
                                    qt[hb:hb + 64, coff:TB],
                                    start=True, stop=True,
                                    tile_position=(hb, 0))
                            if use_mask:
                                for half in range(2):
                                    sl = slice(half * TB + coff,
                                               (half + 1) * TB)
                                    nc.vector.tensor_add(
                                        ps2[:, sl], ps2[:, sl],
                                        mtiles[sig][:, coff:TB])
                            pt = ptpool.tile([128, 1024], BF16, tag="pt",
                                             name=f"P{T}p{p}s{sig}")
                            nc.scalar.activation(pt[:, coff:1024],
                                                 ps2[:, coff:1024],
                                                 mybir.ActivationFunctionType.Exp)
                            if causal and j >= 0:
                                for half in range(2):
                                    sl = slice(half * TB + j * 128,
                                               half * TB + (j + 1) * 128)
                                    nc.vector.tensor_mul(pt[:, sl], pt[:, sl],
                                                         tri_sb[:])
                            pts.append(pt)
                            for half in range(2):
                                h = 2 * p + half
                                nc.tensor.matmul(
                                    po[half][:, coff:TB],
                                    vaug[sig][:, h * 65:h * 65 + 128],
                                    pt[:, half * TB + coff:(half + 1) * TB],
                                    start=(sig == 0), stop=(sig == nsig - 1))
                        # normalization: copy out of PSUM, pair-batched
                        # reciprocal via a DRAM [64,16] reshape, broadcast,
                        # Pool-engine multiply into attnT (odd heads need a
                        # partition-shift DMA via a staging tile).
                        pocs = []
                        for half in range(2):
                            h = 2 * p + half
                            poc = npool.tile([65, TB], F32, tag="poc",
                                             name=f"poc{T}h{h}", bufs=6)
                            nc.vector.tensor_copy(poc[:], po[half][0:65, :])
                            nc.sync.dma_start(d4[h:h + 1, :], poc[64:65, :])
                            pocs.append(poc)
                        rsb = npool.tile([64, 16], F32, tag="rsb",
                                         name=f"rsb{T}p{p}", bufs=4)
                        nc.sync.dma_start(
                            rsb[:], bass.AP(tensor=d4[:].tensor,
                                            offset=d4[:].offset + p * 2 * TB,
                                            ap=[[16, 64], [1, 16]]))
                        rrec = npool.tile([64, 16], F32, tag="rrec",
                                          name=f"rrec{T}p{p}", bufs=4)
                        nc.vector.reciprocal(rrec[:], rsb[:])
                        dr2 = drpool.tile([2, TB], F32, tag="dr2",
                                          name=f"dr2{T}p{p}")
                        nc.sync.dma_start(
                            bass.AP(tensor=dr2[:].tensor, offset=dr2[:].offset,
                                    ap=[[16, 64], [1, 16]]), rrec[:])
                        for half in range(2):
                            rcp = npool.tile([64, TB], F32, tag="rcp",
                                             name=f"rcp{T}p{p}h{half}", bufs=6)
                            bcast = bass.AP(tensor=dr2[:].tensor,
                                            offset=dr2[:].offset + half * TB,
                                            ap=[[0, 64], [1, TB]])
                            nc.sync.dma_start(rcp[:], bcast)
                            strip_norms.setdefault(T, []).append(
                                (pocs[half], rcp, p, half, T))
                        if p < len(fillers) and fillers[p] is not None:
                            fillers[p]()
                    for f in fillers[2:]:
                        if f is not None:
                            f()

                def drain_norms(T, eng=None):
                    eng = eng or nc.gpsimd
                    for poc, rcp, ht, odd, _T in strip_norms.pop(T, []):
                        tcols = slice(_T * TB, (_T + 1) * TB)
                        if not odd:
                            eng.tensor_mul(attnT[ht][0:64, tcols],
                                           poc[0:64, :], rcp[:])
                        else:
                            stag = npool.tile([64, TB], F16, tag="stag",
                                              name=f"stag{_T}x{ht}", bufs=4)
                            eng.tensor_mul(stag[:], poc[0:64, :], rcp[:])
                            nc.sync.dma_start(attnT[ht][64:128, tcols], stag[:])

                def out_proj(taus):
                    for tau in taus:
                        ps = ps_sm.tile([128, EMBED], F32, tag="sm",
                                         name=f"ps_o{tau}")
                        for c in range(2):
                            nc.tensor.matmul(
                                ps[:], attnT[c][:, tau * 128:(tau + 1) * 128],
                                wo_sb[c], start=(c == 0), stop=(c == 1))
                        osb = opool.tile([128, EMBED], F16, tag="osb",
                                         name=f"osb{tau}")
                        nc.vector.tensor_copy(osb[:], ps[:])
                        nc.sync.dma_start(outp[tau * 128:(tau + 1) * 128, :],
                                          osb[:])

                strip_norms = {}

                # --- prologue: strip-0 data prioritized, then remainders ---
                load_w("qc")
                load_w("qs")
                load_x_part("q", 0, TB)
                load_tab_part("cq", 0, TB)
                load_tab_part("sq", 0, TB)
                proj_qk("q", 0, load_x_tb("q", 0), load_tab("cq", 0),
                        load_tab("sq", 0), qTt, es_list=(0,))
                load_w("kc")
                load_w("ks")
                load_x_part("k", 0, TB)
                load_tab_part("ck", 0, TB)
                load_tab_part("sk", 0, TB)
                proj_qk("k", 0, load_x_tb("k", 0), load_tab("ck", 0),
                        load_tab("sk", 0), kTt, es_list=(0,))
                load_w("v")
                load_x_part("v", 0, TB)
                proj_v(0, load_x_tb("v", 0))
                proj_qk("q", 0, load_x_tb("q", 0), load_tab("cq", 0),
                        load_tab("sq", 0), qTt, es_list=(1,))
                proj_qk("k", 0, load_x_tb("k", 0), load_tab("ck", 0),
                        load_tab("sk", 0), kTt, es_list=(1,))
                load_x_part("q", TB, L)
                load_tab_part("cq", TB, L)
                load_tab_part("sq", TB, L)
                load_x_part("k", TB, L)
                load_tab_part("ck", TB, L)
                load_tab_part("sk", TB, L)
                load_x_part("v", TB, L)
                wot = consts.tile([128, 2 * EMBED], BF16, tag="wo",
                                  name="wo", bufs=2)
                nc.sync.dma_start(
                    wot[:], bass.AP(tensor=woT[:].tensor, offset=woT[:].offset,
                                    ap=[[512, 128], [65536, 2], [1, 512]]))
                wo_sb = [wot[:, c * EMBED:(c + 1) * EMBED] for c in range(2)]

                if prev_tail is not None:
                    prev_tail()

                def mk_proj(nm, tb):
                    def f():
                        proj_qk(nm, tb, load_x_tb(nm, tb),
                                load_tab("c" + nm, tb), load_tab("s" + nm, tb),
                                qTt if nm == "q" else kTt)
                    return f

                def mk_projv(tb):
                    return lambda: proj_v(tb, load_x_tb("v", tb))

                def mk_drain_out(tb, eng=None):
                    def f():
                        drain_norms(tb, eng)
                        out_proj(range(tb * 4, (tb + 1) * 4))
                    return f

                for tb in range(NSTRIP):
                    fill = []
                    if tb + 1 < NSTRIP:
                        fill = [mk_proj("q", tb + 1), mk_proj("k", tb + 1),
                                mk_projv(tb + 1)]
                    if tb >= 1:
                        fill.append(mk_drain_out(tb - 1))
                    flash_strip(tb, fill)
                return mk_drain_out(NSTRIP - 1)

            if reps > 1 and reps <= 4:
                # straight-line repetition (for TimelineSim marginal analysis)
                tail = None
                for _ in range(reps):
                    tail = body(tail)
                tail()
            elif reps > 1:
                unroll = 4 if reps % 4 == 0 else 1
                with tc.For_i(0, reps // unroll, 1,
                              hint_engines=(mybir.EngineType.PE,
                                            mybir.EngineType.Activation,
                                            mybir.EngineType.DVE,
                                            mybir.EngineType.SP,
                                            mybir.EngineType.Pool)):
                    tail = None
                    for _ in range(unroll):
                        tail = body(tail)
                    tail()
            else:
                body()()

    nc.compile()
    return nc


_PROGRAM_CACHE = {}


def get_program(causal: bool, use_mask: bool, has_bias: bool, reps: int = 1):
    key = (causal, use_mask, has_bias, reps)
    if key not in _PROGRAM_CACHE:
        _PROGRAM_CACHE[key] = _build_program(causal, use_mask, has_bias, reps)
    return _PROGRAM_CACHE[key]


def _prep_in_maps(query, key, value, key_padding_mask, attn_mask,
                  Wq, bq, Wk, bk, Wv, bv, Wo, bo, use_mask, has_bias):
    """Build the 8 per-core input dicts."""
    import ml_dtypes
    cq, sq, ck, sk = _xpos_tables()
    tri = np.where(np.arange(128)[None, :] >= np.arange(128)[:, None],
                   np.float16(1.0), np.float16(0.0)).astype(np.float16)

    def aug_x(x):
        a = np.empty((513, L), np.float16)
        a[0:512] = np.asarray(x, np.float32).T.astype(np.float16)
        a[512] = np.float16(1.0)
        return a

    xqTs = [aug_x(query[b]) for b in range(B)]
    xkTs = [aug_x(key[b]) for b in range(B)]
    xvTs = [aug_x(value[b]) for b in range(B)]

    masks = None
    if use_mask:
        am = np.asarray(attn_mask, np.float32)
        kp = np.asarray(key_padding_mask)
        masks = []
        for b in range(B):
            m = am.copy()
            if kp[b].any():
                m = m + np.where(kp[b], np.float32(-1e30),
                                 np.float32(0.0))[None, :]
            masks.append(np.ascontiguousarray(m.T.astype(np.float32)))

    Wq = np.asarray(Wq, np.float32); bq = np.asarray(bq, np.float32)
    Wk = np.asarray(Wk, np.float32); bk = np.asarray(bk, np.float32)
    Wv = np.asarray(Wv, np.float32); bv = np.asarray(bv, np.float32)
    Wo = np.asarray(Wo, np.float32)

    in_maps = []
    for core in range(N_CORES):
        b, hg = core // 2, core % 2
        hs = hg * HPC
        idx_p = np.concatenate(
            [hs * HD + hl * HD + _PERM64 for hl in range(HPC)])
        # sin-projection rows: within each head's 64-block, row r <- r XOR 32
        xor = (np.arange(256).reshape(HPC, HD)[:, (np.arange(HD) ^ 32)]
               ).reshape(-1)
        idx_s = idx_p[xor]
        idx_v = hs * HD + np.arange(HPC * HD)

        def aug_w(W, bias, idx):
            a = np.empty((513, 256), np.float32)
            a[0:512] = np.ascontiguousarray(W[idx, :].T)
            a[512] = bias[idx]
            return a.astype(ml_dtypes.bfloat16)

        m = {
            "xqT": xqTs[b], "xkT": xkTs[b], "xvT": xvTs[b],
            "wqcT": aug_w(Wq, bq, idx_p),
            "wqsT": aug_w(Wq, bq, idx_s),
            "wkcT": aug_w(Wk, bk, idx_p),
            "wksT": aug_w(Wk, bk, idx_s),
            "wvT": aug_w(Wv, bv, idx_v),
            "woT": np.ascontiguousarray(Wo[:, idx_v].T).astype(ml_dtypes.bfloat16),
            "cq": cq, "sq": sq, "ck": ck, "sk": sk,
            "tri": tri,
        }
        if use_mask:
            m["maskT"] = masks[b]
        in_maps.append(m)
    return in_maps


def classify_mask(attn_mask, key_padding_mask):
    am = np.asarray(attn_mask, np.float32)
    kp = np.asarray(key_padding_mask)
    if not kp.any():
        causal = np.where(
            np.tril(np.ones((L, L), bool)), np.float32(0.0),
            np.float32(NEG)).astype(np.float32)
        if np.array_equal(am, causal):
            return True, False
        if not am.any():
            return False, False
    return False, True


def kernel(query, key, value, key_padding_mask, attn_mask,
           Wq, bq, Wk, bk, Wv, bv, Wo, bo):
    causal, use_mask = classify_mask(attn_mask, key_padding_mask)
    has_bias = bool(np.asarray(bq).any() or np.asarray(bk).any()
                    or np.asarray(bv).any())
    nc = get_program(causal, use_mask, has_bias, reps=1)
    in_maps = _prep_in_maps(query, key, value, key_padding_mask, attn_mask,
                            Wq, bq, Wk, bk, Wv, bv, Wo, bo, use_mask, has_bias)
    res = run_bass_kernel_spmd(nc, in_maps, list(range(N_CORES)))
    bo = np.asarray(bo, np.float32)
    out = np.empty((B, L, EMBED), np.float32)
    for b in range(B):
        out[b] = (res.results[2 * b]["outp"].astype(np.float32)
                  + res.results[2 * b + 1]["outp"].astype(np.float32)
                  + bo[None, :])
    return out


# revision 33
# speedup vs baseline: 2.6227x; 1.0923x over previous
"""Bass/Trainium2 kernel for nn_BerpXposMultiHeadedAttention (8-core SPMD).

Sharding: data-parallel over batch (4 batches x 2 cores) x tensor-parallel over
heads (4 heads per core).  Each core computes its 4 heads of flash-style xpos
attention for its batch plus the row-sharded partial out-projection; the host
sums the two partials per batch (the "all-reduce") and adds the output bias.

Design notes (v2, rebalanced from HW slope measurements + TimelineSim):
- All matmul operands are 16-bit: bf16 weights (stationary) x fp16 activations,
  fp16 q/k for QK^T (FWL weight loads), bf16 probabilities for P@V.  fp32
  PSUM accumulation throughout keeps the softmax inputs accurate.
- xpos rotation via dual projection (cos-path and sin-path with host-permuted
  weight rows); the elementwise combine runs as two DVE muls (fp16 tables) and
  one gpsimd (Pool) add, keeping DVE off the critical path.
- Causal fast path trims score matmuls, exp consumption, and P@V moving
  columns to the exact 128-block diagonal; only the diagonal 128x128 block
  needs a triangle mask (DVE add of a -1e9 upper-triangle on PSUM).
- Softmax normalization: P@V accumulates an extra ones-row per head (PSUM row
  64 = sumexp).  reciprocal_approx_fast on the [1,512] denominator row, DRAM
  round-trip broadcast to [64,512], multiply on the Pool engine.  No 6-cpe
  DVE reciprocal, no [64,512] DVE multiply.
- Emission interleaves projection strips with flash strips so all engines ramp
  early; the causal fast path skips above-diagonal blocks entirely.
"""

import sys

sys.path.insert(0, "/opt/trn_rl_repo")

import contextlib

import numpy as np

import concourse.bacc as bacc
import concourse.bass as bass
import concourse.tile as tile
from concourse import mybir
from concourse.bass_utils import run_bass_kernel_spmd

# Problem constants (hardcoded per the task contract).
B = 4
L = 2048
EMBED = 512
HEADS = 8
HD = 64
SCALE_BASE = 512
NEG = -1e9
N_CORES = 8
HPC = 4           # heads per core
TB = 512          # t-block (strip) width
NT = L // 128     # 16 t-chunks
NS = L // 128     # 16 s-chunks
NSTRIP = L // TB  # 4 strips
VW = 328          # v_aug tile width (4 heads x 65 + 68 junk tail)

F32 = mybir.dt.float32
F32R = mybir.dt.float32r
F16 = mybir.dt.float16
BF16 = mybir.dt.bfloat16

# Deinterleave permutation of a 64-wide head dim: evens then odds.
_PERM64 = np.concatenate([np.arange(0, HD, 2), np.arange(1, HD, 2)])


def _xpos_tables():
    """Host-side xpos cos/sin tables in the permuted [d, t] layout.

    Returns (cq, sq, ck, sk), each [128, L] float16 (two heads' worth of rows,
    identical per head).  The 1/sqrt(HD) score scale is folded into the q pair.
    """
    d = HD
    base = ((np.arange(0, d, 2, dtype=np.float32) + np.float32(0.4 * d))
            / np.float32(1.4 * d)).astype(np.float32)                    # [32]
    min_pos = -(L // 2)
    power = (np.arange(min_pos, L + min_pos, dtype=np.float32)
             / np.float32(SCALE_BASE))                                   # [L]
    scale = (base[None, :] ** power[:, None]).astype(np.float32)         # [L, 32]
    half = d // 2
    inv_freq = (1.0 / (10000.0 ** (np.arange(half, dtype=np.float32) / half))
                ).astype(np.float32)
    sinusoid = np.arange(L, dtype=np.float32)[:, None] * inv_freq[None, :]
    sin = np.sin(sinusoid).astype(np.float32)
    cos = np.cos(sinusoid).astype(np.float32)

    def pack(cs, ss, fold):
        cs = (cs * fold).astype(np.float32)
        ss = (ss * fold).astype(np.float32)
        # permuted layout: rows 0:32 <- even orig dims, rows 32:64 <- odd.
        cos_p = np.concatenate([cs.T, cs.T], axis=0)      # [64, L]
        sin_p = np.concatenate([-ss.T, ss.T], axis=0)     # [64, L]
        return (np.concatenate([cos_p, cos_p], axis=0).astype(np.float16),
                np.concatenate([sin_p, sin_p], axis=0).astype(np.float16))

    inv_scale = (1.0 / scale).astype(np.float32)
    cq, sq = pack(cos * scale, sin * scale, np.float32(HD ** -0.5))
    ck, sk = pack(cos * inv_scale, sin * inv_scale, np.float32(1.0))
    return cq, sq, ck, sk


def _build_program(causal: bool, use_mask: bool, has_bias: bool, reps: int = 1):
    nc = bacc.Bacc("TRN2", target_bir_lowering=False, debug=False,
                   num_devices=N_CORES)

    # ---- DRAM I/O -------------------------------------------------------
    xqT = nc.dram_tensor("xqT", [513, L], F16, kind="ExternalInput")
    xkT = nc.dram_tensor("xkT", [513, L], F16, kind="ExternalInput")
    xvT = nc.dram_tensor("xvT", [513, L], F16, kind="ExternalInput")
    wqcT = nc.dram_tensor("wqcT", [513, 256], BF16, kind="ExternalInput")
    wqsT = nc.dram_tensor("wqsT", [513, 256], BF16, kind="ExternalInput")
    wkcT = nc.dram_tensor("wkcT", [513, 256], BF16, kind="ExternalInput")
    wksT = nc.dram_tensor("wksT", [513, 256], BF16, kind="ExternalInput")
    wvT = nc.dram_tensor("wvT", [513, 256], BF16, kind="ExternalInput")
    woT = nc.dram_tensor("woT", [256, EMBED], BF16, kind="ExternalInput")
    cqD = nc.dram_tensor("cq", [128, L], F16, kind="ExternalInput")
    sqD = nc.dram_tensor("sq", [128, L], F16, kind="ExternalInput")
    ckD = nc.dram_tensor("ck", [128, L], F16, kind="ExternalInput")
    skD = nc.dram_tensor("sk", [128, L], F16, kind="ExternalInput")
    triD = nc.dram_tensor("tri", [128, 128], F16, kind="ExternalInput")
    maskD = None
    if use_mask:
        maskD = nc.dram_tensor("maskT", [L, L], F32, kind="ExternalInput")
    outp = nc.dram_tensor("outp", [L, EMBED], F16, kind="ExternalOutput")

    xin = {"q": xqT, "k": xkT, "v": xvT}
    win = {"qc": wqcT, "qs": wqsT, "kc": wkcT, "ks": wksT, "v": wvT}
    tabin = {"cq": cqD, "sq": sqD, "ck": ckD, "sk": skD}

    with tile.TileContext(nc) as tc:
        with contextlib.ExitStack() as ctx:
            consts = ctx.enter_context(tc.tile_pool(name="consts", bufs=1))
            xpool = ctx.enter_context(tc.tile_pool(name="xpool", bufs=1))
            wpool = ctx.enter_context(tc.tile_pool(name="wpool", bufs=1))
            qkpool = ctx.enter_context(tc.tile_pool(name="qkpool", bufs=1))
            tabpool = ctx.enter_context(tc.tile_pool(name="tabpool", bufs=1))
            vpool = ctx.enter_context(tc.tile_pool(name="vpool", bufs=NS + 4))
            tmp = ctx.enter_context(tc.tile_pool(name="tmp", bufs=3))
            ptpool = ctx.enter_context(tc.tile_pool(name="ptpool", bufs=10))
            npool = ctx.enter_context(tc.tile_pool(name="npool", bufs=3))
            opool = ctx.enter_context(tc.tile_pool(name="opool", bufs=2))
            drpool = ctx.enter_context(
                tc.tile_pool(name="drpool", bufs=3, space="DRAM"))
            mpool = None
            if use_mask:
                mpool = ctx.enter_context(tc.tile_pool(name="mpool", bufs=NS + 2))
            ps_s = ctx.enter_context(tc.tile_pool(name="ps_s", bufs=2, space="PSUM"))
            ps_sm = ctx.enter_context(tc.tile_pool(name="ps_sm", bufs=2, space="PSUM"))
            ps_pv = ctx.enter_context(tc.tile_pool(name="ps_pv", bufs=2, space="PSUM"))

            def body(prev_tail=None):
                # ---- stage-0 small constants ----
                ones_sb = None
                if has_bias:
                    ones_sb = consts.tile([1, L], F16, tag="ones")
                    nc.sync.dma_start(ones_sb[:], xqT[512:513, :])
                tri_sb = consts.tile([128, 128], F16, tag="tri", bufs=2)
                if causal:
                    nc.sync.dma_start(tri_sb[:], triD[:])

                w_sb = {}

                def load_w(nm):
                    wt = wpool.tile([128, 1024], BF16, tag=f"w{nm}",
                                    name=f"w{nm}")
                    wd = win[nm]
                    nc.sync.dma_start(
                        wt[:], bass.AP(tensor=wd[:].tensor, offset=wd[:].offset,
                                       ap=[[256, 128], [32768, 4], [1, 256]]))
                    chunks = [wt[:, c * 256:(c + 1) * 256] for c in range(4)]
                    bt = None
                    if has_bias:
                        bt = wpool.tile([1, 256], BF16, tag=f"w{nm}b",
                                        name=f"w{nm}b")
                        nc.sync.dma_start(bt[:], win[nm][512:513, :])
                    w_sb[nm] = (chunks, bt)

                attnT = [consts.tile([128, L], F16, tag=f"attnT{c}",
                                     name=f"attnT{c}", bufs=2) for c in range(2)]

                qTt = [[None] * NSTRIP for _ in range(2)]  # [e][tb]
                kTt = [[None] * NSTRIP for _ in range(2)]
                vaug = [None] * NS

                xfull = {}

                def load_x_part(nm, lo, hi):
                    if nm not in xfull:
                        xfull[nm] = [xpool.tile([128, L], F16, tag=f"x{nm}{c}",
                                                name=f"x{nm}{c}")
                                     for c in range(4)]
                    for c in range(4):
                        nc.sync.dma_start(
                            xfull[nm][c][:, lo:hi],
                            xin[nm][c * 128:(c + 1) * 128, lo:hi])

                def load_x_tb(nm, tb):
                    return [t[:, tb * TB:(tb + 1) * TB] for t in xfull[nm]]

                tabfull = {}

                def load_tab_part(nm, lo, hi):
                    if nm not in tabfull:
                        tabfull[nm] = tabpool.tile([128, L], F16, tag=nm,
                                                   name=f"{nm}full")
                    nc.sync.dma_start(tabfull[nm][:, lo:hi],
                                      tabin[nm][:, lo:hi])

                def load_tab(nm, tb):
                    return tabfull[nm][:, tb * TB:(tb + 1) * TB]

                def proj_qk(nm, tb, xs, ctab, stab, dst, es_list=(0, 1)):
                    wc, wcb = w_sb[nm + "c"]
                    ws, wsb = w_sb[nm + "s"]
                    on = (ones_sb[:, tb * TB:(tb + 1) * TB]
                          if has_bias else None)
                    for e in es_list:
                        es = slice(e * 128, (e + 1) * 128)
                        psc = ps_sm.tile([128, TB], F32, tag="sm",
                                         name=f"ps_{nm}c{e}_{tb}")
                        pss = ps_sm.tile([128, TB], F32, tag="sm",
                                         name=f"ps_{nm}s{e}_{tb}")
                        for c in range(4):
                            nc.tensor.matmul(psc[:], wc[c][:, es], xs[c],
                                             start=(c == 0),
                                             stop=(c == 3 and not has_bias))
                        if has_bias:
                            nc.tensor.matmul(psc[:], wcb[:, es], on,
                                             start=False, stop=True)
                        for c in range(4):
                            nc.tensor.matmul(pss[:], ws[c][:, es], xs[c],
                                             start=(c == 0),
                                             stop=(c == 3 and not has_bias))
                        if has_bias:
                            nc.tensor.matmul(pss[:], wsb[:, es], on,
                                             start=False, stop=True)
                        t1 = tmp.tile([128, TB], F32, tag="t1",
                                      name=f"t1{nm}{e}{tb}")
                        nc.vector.tensor_mul(t1[:], psc[:], ctab)
                        t2 = tmp.tile([128, TB], F32, tag="t2",
                                      name=f"t2{nm}{e}{tb}")
                        nc.vector.tensor_mul(t2[:], pss[:], stab)
                        ot = qkpool.tile([128, TB], F16, tag=f"{nm}T{e}_{tb}",
                                         name=f"{nm}T{e}_{tb}")
                        nc.gpsimd.tensor_add(ot[:], t1[:], t2[:])
                        dst[e][tb] = ot

                def proj_v(tb, xs):
                    wv, wvb = w_sb["v"]
                    for j in range(4):
                        si = tb * 4 + j
                        js = slice(j * 128, (j + 1) * 128)
                        ps = ps_sm.tile([128, 256], F32, tag="sm",
                                        name=f"ps_v{si}")
                        for c in range(4):
                            nc.tensor.matmul(ps[:], xs[c][:, js], wv[c][:],
                                             start=(c == 0),
                                             stop=(c == 3 and not has_bias))
                        if has_bias:
                            nc.tensor.matmul(
                                ps[:], ones_sb[:, si * 128:(si + 1) * 128],
                                wvb[:], start=False, stop=True)
                        va = vpool.tile([128, VW], BF16, tag="vaug",
                                        name=f"vaug{si}")
                        va3 = va[:, 0:HPC * 65].rearrange("p (h c) -> p h c", c=65)
                        nc.vector.tensor_copy(
                            va3[:, :, 0:64],
                            ps[:].rearrange("p (h d) -> p h d", d=64))
                        nc.vector.memset(va3[:, :, 64:65], 1.0)
                        vaug[si] = va

                def flash_strip(T, fillers=()):
                    nsig = 4 * T + 4 if causal else NS
                    mtiles = None
                    if use_mask:
                        mtiles = []
                        for si in range(nsig):
                            mt = mpool.tile([128, TB], F32, tag="mask",
                                            name=f"m{T}_{si}")
                            nc.sync.dma_start(
                                mt[:], maskD[si * 128:(si + 1) * 128,
                                             T * TB:(T + 1) * TB])
                            mtiles.append(mt)
                    d4 = drpool.tile([4, TB], F32, tag="d4", name=f"d4{T}")
                    for p in range(2):
                        # heads A=2p (rows 0:64) and B=2p+1 (rows 64:128) of
                        # the pair-tile run as concurrent row-tiled matmuls
                        # into the two halves of a shared [128,1024] tile.
                        po = [ps_pv.tile([128, TB], F32, tag="pv",
                                         name=f"po{T}p{p}h{half}")
                              for half in range(2)]
                        pts = []
                        coffs = {}
                        for sig in range(nsig):
                            j = sig - 4 * T
                            coff = j * 128 if (causal and j > 0) else 0
                            coffs[sig] = coff
                            ps2 = ps_s.tile([128, 1024], F32, tag="s",
                                            name=f"S{T}p{p}s{sig}")
                            kt = kTt[p][sig // 4]
                            qt = qTt[p][T]
                            scols = slice((sig % 4) * 128, (sig % 4 + 1) * 128)
                            for half, hb in ((0, 0), (1, 64)):
                                nc.tensor.matmul(
                                    ps2[:, half * TB + coff:(half + 1) * TB],
                                    kt[hb:hb + 64, scols],
                                    qt[hb:hb + 64, coff:TB],
                                    start=True, stop=True,
                                    tile_position=(hb, 0))
                            if use_mask:
                                for half in range(2):
                                    sl = slice(half * TB + coff,
                                               (half + 1) * TB)
                                    nc.vector.tensor_add(
                                        ps2[:, sl], ps2[:, sl],
                                        mtiles[sig][:, coff:TB])
                            pt = ptpool.tile([128, 1024], BF16, tag="pt",
                                             name=f"P{T}p{p}s{sig}")
                            nc.scalar.activation(pt[:, coff:1024],
                                                 ps2[:, coff:1024],
                                                 mybir.ActivationFunctionType.Exp)
                            if causal and j >= 0:
                                for half in range(2):
                                    sl = slice(half * TB + j * 128,
                                               half * TB + (j + 1) * 128)
                                    nc.vector.tensor_mul(pt[:, sl], pt[:, sl],
                                                         tri_sb[:])
                            pts.append(pt)
                            for half in range(2):
                                h = 2 * p + half
                                nc.tensor.matmul(
                                    po[half][:, coff:TB],
                                    vaug[sig][:, h * 65:h * 65 + 128],
                                    pt[:, half * TB + coff:(half + 1) * TB],
                                    start=(sig == 0), stop=(sig == nsig - 1))
                        # normalization: copy out of PSUM, pair-batched
                        # reciprocal via a DRAM [64,16] reshape, broadcast,
                        # Pool-engine multiply into attnT (odd heads need a
                        # partition-shift DMA via a staging tile).
                        pocs = []
                        for half in range(2):
                            h = 2 * p + half
                            poc = npool.tile([65, TB], F32, tag="poc",
                                             name=f"poc{T}h{h}", bufs=6)
                            nc.vector.tensor_copy(poc[:], po[half][0:65, :])
                            nc.sync.dma_start(d4[h:h + 1, :], poc[64:65, :])
                            pocs.append(poc)
                        rsb = npool.tile([64, 16], F32, tag="rsb",
                                         name=f"rsb{T}p{p}", bufs=4)
                        nc.sync.dma_start(
                            rsb[:], bass.AP(tensor=d4[:].tensor,
                                            offset=d4[:].offset + p * 2 * TB,
                                            ap=[[16, 64], [1, 16]]))
                        rrec = npool.tile([64, 16], F32, tag="rrec",
                                          name=f"rrec{T}p{p}", bufs=4)
                        nc.vector.reciprocal(rrec[:], rsb[:])
                        dr2 = drpool.tile([2, TB], F32, tag="dr2",
                                          name=f"dr2{T}p{p}")
                        nc.sync.dma_start(
                            bass.AP(tensor=dr2[:].tensor, offset=dr2[:].offset,
                                    ap=[[16, 64], [1, 16]]), rrec[:])
                        for half in range(2):
                            rcp = npool.tile([64, TB], F32, tag="rcp",
                                             name=f"rcp{T}p{p}h{half}", bufs=6)
                            bcast = bass.AP(tensor=dr2[:].tensor,
                                            offset=dr2[:].offset + half * TB,
                                            ap=[[0, 64], [1, TB]])
                            nc.sync.dma_start(rcp[:], bcast)
                            strip_norms.setdefault(T, []).append(
                                (pocs[half], rcp, p, half, T))
                        if p < len(fillers) and fillers[p] is not None:
                            fillers[p]()
                    for f in fillers[2:]:
                        if f is not None:
                            f()

                def drain_norms(T, eng=None):
                    eng = eng or nc.gpsimd
                    for poc, rcp, ht, odd, _T in strip_norms.pop(T, []):
                        tcols = slice(_T * TB, (_T + 1) * TB)
                        if not odd:
                            eng.tensor_mul(attnT[ht][0:64, tcols],
                                           poc[0:64, :], rcp[:])
                        else:
                            stag = npool.tile([64, TB], F16, tag="stag",
                                              name=f"stag{_T}x{ht}", bufs=4)
                            eng.tensor_mul(stag[:], poc[0:64, :], rcp[:])
                            nc.sync.dma_start(attnT[ht][64:128, tcols], stag[:])

                def out_proj(taus):
                    for tau in taus:
                        ps = ps_sm.tile([128, EMBED], F32, tag="sm",
                                         name=f"ps_o{tau}")
                        for c in range(2):
                            nc.tensor.matmul(
                                ps[:], attnT[c][:, tau * 128:(tau + 1) * 128],
                                wo_sb[c], start=(c == 0), stop=(c == 1))
                        osb = opool.tile([128, EMBED], F16, tag="osb",
                                         name=f"osb{tau}")
                        nc.vector.tensor_copy(osb[:], ps[:])
                        nc.sync.dma_start(outp[tau * 128:(tau + 1) * 128, :],
                                          osb[:])

                strip_norms = {}

                # --- prologue: strip-0 data prioritized, then remainders ---
                load_w("qc")
                load_w("qs")
                load_x_part("q", 0, TB)
                load_tab_part("cq", 0, TB)
                load_tab_part("sq", 0, TB)
                proj_qk("q", 0, load_x_tb("q", 0), load_tab("cq", 0),
                        load_tab("sq", 0), qTt, es_list=(0,))
                load_w("kc")
                load_w("ks")
                load_x_part("k", 0, TB)
                load_tab_part("ck", 0, TB)
                load_tab_part("sk", 0, TB)
                proj_qk("k", 0, load_x_tb("k", 0), load_tab("ck", 0),
                        load_tab("sk", 0), kTt, es_list=(0,))
                load_w("v")
                load_x_part("v", 0, TB)
                proj_v(0, load_x_tb("v", 0))
                proj_qk("q", 0, load_x_tb("q", 0), load_tab("cq", 0),
                        load_tab("sq", 0), qTt, es_list=(1,))
                proj_qk("k", 0, load_x_tb("k", 0), load_tab("ck", 0),
                        load_tab("sk", 0), kTt, es_list=(1,))
                load_x_part("q", TB, L)
                load_tab_part("cq", TB, L)
                load_tab_part("sq", TB, L)
                load_x_part("k", TB, L)
                load_tab_part("ck", TB, L)
                load_tab_part("sk", TB, L)
                load_x_part("v", TB, L)
                wot = consts.tile([128, 2 * EMBED], BF16, tag="wo",
                                  name="wo", bufs=2)
                nc.sync.dma_start(
                    wot[:], bass.AP(tensor=woT[:].tensor, offset=woT[:].offset,
                                    ap=[[512, 128], [65536, 2], [1, 512]]))
                wo_sb = [wot[:, c * EMBED:(c + 1) * EMBED] for c in range(2)]

                if prev_tail is not None:
                    prev_tail()

                def mk_proj(nm, tb):
                    def f():
                        proj_qk(nm, tb, load_x_tb(nm, tb),
                                load_tab("c" + nm, tb), load_tab("s" + nm, tb),
                                qTt if nm == "q" else kTt)
                    return f

                def mk_projv(tb):
                    return lambda: proj_v(tb, load_x_tb("v", tb))

                def mk_drain_out(tb, eng=None):
                    def f():
                        drain_norms(tb, eng)
                        out_proj(range(tb * 4, (tb + 1) * 4))
                    return f

                for tb in range(NSTRIP):
                    fill = []
                    if tb + 1 < NSTRIP:
                        fill = [mk_proj("q", tb + 1), mk_proj("k", tb + 1),
                                mk_projv(tb + 1)]
                    if tb >= 1:
                        fill.append(mk_drain_out(tb - 1))
                    flash_strip(tb, fill)
                return mk_drain_out(NSTRIP - 1)

            if reps > 1 and reps <= 4:
                # straight-line repetition (for TimelineSim marginal analysis)
                tail = None
                for _ in range(reps):
                    tail = body(tail)
                tail()
            elif reps > 1:
                unroll = 4 if reps % 4 == 0 else 1
                with tc.For_i(0, reps // unroll, 1,
                              staggered_reset=True,
                              hint_engines=(mybir.EngineType.PE,
                                            mybir.EngineType.Activation,
                                            mybir.EngineType.DVE,
                                            mybir.EngineType.SP,
                                            mybir.EngineType.Pool)):
                    tail = None
                    for _ in range(unroll):
                        tail = body(tail)
                    tail()
            else:
                body()()

    nc.compile()
    return nc


_PROGRAM_CACHE = {}


def get_program(causal: bool, use_mask: bool, has_bias: bool, reps: int = 1):
    key = (causal, use_mask, has_bias, reps)
    if key not in _PROGRAM_CACHE:
        _PROGRAM_CACHE[key] = _build_program(causal, use_mask, has_bias, reps)
    return _PROGRAM_CACHE[key]


def _prep_in_maps(query, key, value, key_padding_mask, attn_mask,
                  Wq, bq, Wk, bk, Wv, bv, Wo, bo, use_mask, has_bias):
    """Build the 8 per-core input dicts."""
    import ml_dtypes
    cq, sq, ck, sk = _xpos_tables()
    tri = np.where(np.arange(128)[None, :] >= np.arange(128)[:, None],
                   np.float16(1.0), np.float16(0.0)).astype(np.float16)

    def aug_x(x):
        a = np.empty((513, L), np.float16)
        a[0:512] = np.asarray(x, np.float32).T.astype(np.float16)
        a[512] = np.float16(1.0)
        return a

    xqTs = [aug_x(query[b]) for b in range(B)]
    xkTs = [aug_x(key[b]) for b in range(B)]
    xvTs = [aug_x(value[b]) for b in range(B)]

    masks = None
    if use_mask:
        am = np.asarray(attn_mask, np.float32)
        kp = np.asarray(key_padding_mask)
        masks = []
        for b in range(B):
            m = am.copy()
            if kp[b].any():
                m = m + np.where(kp[b], np.float32(-1e30),
                                 np.float32(0.0))[None, :]
            masks.append(np.ascontiguousarray(m.T.astype(np.float32)))

    Wq = np.asarray(Wq, np.float32); bq = np.asarray(bq, np.float32)
    Wk = np.asarray(Wk, np.float32); bk = np.asarray(bk, np.float32)
    Wv = np.asarray(Wv, np.float32); bv = np.asarray(bv, np.float32)
    Wo = np.asarray(Wo, np.float32)

    in_maps = []
    for core in range(N_CORES):
        b, hg = core // 2, core % 2
        hs = hg * HPC
        idx_p = np.concatenate(
            [hs * HD + hl * HD + _PERM64 for hl in range(HPC)])
        # sin-projection rows: within each head's 64-block, row r <- r XOR 32
        xor = (np.arange(256).reshape(HPC, HD)[:, (np.arange(HD) ^ 32)]
               ).reshape(-1)
        idx_s = idx_p[xor]
        idx_v = hs * HD + np.arange(HPC * HD)

        def aug_w(W, bias, idx):
            a = np.empty((513, 256), np.float32)
            a[0:512] = np.ascontiguousarray(W[idx, :].T)
            a[512] = bias[idx]
            return a.astype(ml_dtypes.bfloat16)

        m = {
            "xqT": xqTs[b], "xkT": xkTs[b], "xvT": xvTs[b],
            "wqcT": aug_w(Wq, bq, idx_p),
            "wqsT": aug_w(Wq, bq, idx_s),
            "wkcT": aug_w(Wk, bk, idx_p),
            "wksT": aug_w(Wk, bk, idx_s),
            "wvT": aug_w(Wv, bv, idx_v),
            "woT": np.ascontiguousarray(Wo[:, idx_v].T).astype(ml_dtypes.bfloat16),
            "cq": cq, "sq": sq, "ck": ck, "sk": sk,
            "tri": tri,
        }
        if use_mask:
            m["maskT"] = masks[b]
        in_maps.append(m)
    return in_maps


def classify_mask(attn_mask, key_padding_mask):
    am = np.asarray(attn_mask, np.float32)
    kp = np.asarray(key_padding_mask)
    if not kp.any():
        causal = np.where(
            np.tril(np.ones((L, L), bool)), np.float32(0.0),
            np.float32(NEG)).astype(np.float32)
        if np.array_equal(am, causal):
            return True, False
        if not am.any():
            return False, False
    return False, True


def kernel(query, key, value, key_padding_mask, attn_mask,
           Wq, bq, Wk, bk, Wv, bv, Wo, bo):
    causal, use_mask = classify_mask(attn_mask, key_padding_mask)
    has_bias = bool(np.asarray(bq).any() or np.asarray(bk).any()
                    or np.asarray(bv).any())
    nc = get_program(causal, use_mask, has_bias, reps=1)
    in_maps = _prep_in_maps(query, key, value, key_padding_mask, attn_mask,
                            Wq, bq, Wk, bk, Wv, bv, Wo, bo, use_mask, has_bias)
    res = run_bass_kernel_spmd(nc, in_maps, list(range(N_CORES)))
    bo = np.asarray(bo, np.float32)
    out = np.empty((B, L, EMBED), np.float32)
    for b in range(B):
        out[b] = (res.results[2 * b]["outp"].astype(np.float32)
                  + res.results[2 * b + 1]["outp"].astype(np.float32)
                  + bo[None, :])
    return out


# revision 44
# speedup vs baseline: 3.0181x; 1.1507x over previous
"""Bass/Trainium2 kernel for nn_BerpXposMultiHeadedAttention (8-core SPMD).

Sharding: data-parallel over batch (4 batches x 2 cores) x tensor-parallel over
heads (4 heads per core).  Each core computes its 4 heads of flash-style xpos
attention for its batch plus the row-sharded partial out-projection; the host
sums the two partials per batch (the "all-reduce") and adds the output bias.

Design notes (measured on HW via reps-slope + TimelineSim occupancy):
- All matmul operands are 16-bit: bf16 weights (stationary) x fp16 activations,
  fp16 q/k for QK^T (enables fast weight loads), bf16 probabilities for P@V.
  fp32 PSUM accumulation keeps softmax inputs accurate (rel err ~2.7e-3).
- Head-PAIRED QK^T: the two heads of a q/k pair-tile run as concurrent
  row-tiled matmuls (tile_position (0,0)/(64,0), K=64 each) writing the two
  halves of a shared [128,1024] score tile -> one exp feeds both heads and
  QK^T costs half the PE time.  Measured ~16us/iter win.
- xpos rotation via a SINGLE projection per q/k: the rotate-half partner is a
  row swap r <-> r^32 in the deinterleaved layout, computed by one matmul
  against a constant 128x128 permutation (saves 3 of 8 proj matmuls and half
  the q/k weight DMA); combine = two DVE muls (fp16 tables) + one Pool add.
- Causal fast path trims score matmuls, exp, and P@V moving columns to the
  exact 128-block diagonal; the diagonal block is masked by a post-exp 0/1
  lower-triangle multiply on bf16 P (off the PSUM critical path).
- Softmax: P@V accumulates a ones-row per head (PSUM row 64 = sumexp);
  per-pair denominators go through a DRAM [64,16]-reshape so one 6-cpe DVE
  reciprocal covers 1024 values in ~100ns, then a stride-0 DMA broadcast and
  a Pool multiply normalize into attnT (odd heads partition-shift via DMA).
- PSUM: scores double-buffered (2x2 banks), 1-bank slots for proj/v/out (2),
  2 po banks = 8.  Single-DMA weight loads ([513,256] -> [128,1024] tiles);
  out-projection staged per strip into one [128,2048] tile -> one DMA.
- Emission: strip-0 loads prioritized; proj of strip T+1, norm-drain and
  out-proj of strip T-1 are emitted as fillers inside flash strip T; each
  body passes its final drain+out_proj into the NEXT body (engines are
  in-order, so tail work must sit behind the next body head in the stream).
- Timing loop: For_i over reps/4 with 4 unrolled bodies and staggered_reset
  (the plain loop ends in an all-engine barrier which serializes iterations).
"""

import sys

sys.path.insert(0, "/opt/trn_rl_repo")

import contextlib

import numpy as np

import concourse.bacc as bacc
import concourse.bass as bass
import concourse.tile as tile
from concourse import mybir
from concourse.bass_utils import run_bass_kernel_spmd

# Problem constants (hardcoded per the task contract).
B = 4
L = 2048
EMBED = 512
HEADS = 8
HD = 64
SCALE_BASE = 512
NEG = -1e9
N_CORES = 8
HPC = 4           # heads per core
TB = 512          # t-block (strip) width
NT = L // 128     # 16 t-chunks
NS = L // 128     # 16 s-chunks
NSTRIP = L // TB  # 4 strips
VW = 328          # v_aug tile width (4 heads x 65 + 68 junk tail)

F32 = mybir.dt.float32
F32R = mybir.dt.float32r
F16 = mybir.dt.float16
BF16 = mybir.dt.bfloat16

# Deinterleave permutation of a 64-wide head dim: evens then odds.
_PERM64 = np.concatenate([np.arange(0, HD, 2), np.arange(1, HD, 2)])


def _xpos_tables():
    """Host-side xpos cos/sin tables in the permuted [d, t] layout.

    Returns (cq, sq, ck, sk), each [128, L] float16 (two heads' worth of rows,
    identical per head).  The 1/sqrt(HD) score scale is folded into the q pair.
    """
    d = HD
    base = ((np.arange(0, d, 2, dtype=np.float32) + np.float32(0.4 * d))
            / np.float32(1.4 * d)).astype(np.float32)                    # [32]
    min_pos = -(L // 2)
    power = (np.arange(min_pos, L + min_pos, dtype=np.float32)
             / np.float32(SCALE_BASE))                                   # [L]
    scale = (base[None, :] ** power[:, None]).astype(np.float32)         # [L, 32]
    half = d // 2
    inv_freq = (1.0 / (10000.0 ** (np.arange(half, dtype=np.float32) / half))
                ).astype(np.float32)
    sinusoid = np.arange(L, dtype=np.float32)[:, None] * inv_freq[None, :]
    sin = np.sin(sinusoid).astype(np.float32)
    cos = np.cos(sinusoid).astype(np.float32)

    def pack(cs, ss, fold):
        cs = (cs * fold).astype(np.float32)
        ss = (ss * fold).astype(np.float32)
        # permuted layout: rows 0:32 <- even orig dims, rows 32:64 <- odd.
        cos_p = np.concatenate([cs.T, cs.T], axis=0)      # [64, L]
        sin_p = np.concatenate([-ss.T, ss.T], axis=0)     # [64, L]
        return (np.concatenate([cos_p, cos_p], axis=0).astype(np.float16),
                np.concatenate([sin_p, sin_p], axis=0).astype(np.float16))

    inv_scale = (1.0 / scale).astype(np.float32)
    cq, sq = pack(cos * scale, sin * scale, np.float32(HD ** -0.5))
    ck, sk = pack(cos * inv_scale, sin * inv_scale, np.float32(1.0))
    return cq, sq, ck, sk


def _build_program(causal: bool, use_mask: bool, has_bias: bool, reps: int = 1):
    nc = bacc.Bacc("TRN2", target_bir_lowering=False, debug=False,
                   num_devices=N_CORES)

    # ---- DRAM I/O -------------------------------------------------------
    xqT = nc.dram_tensor("xqT", [513, L], F16, kind="ExternalInput")
    xkT = nc.dram_tensor("xkT", [513, L], F16, kind="ExternalInput")
    xvT = nc.dram_tensor("xvT", [513, L], F16, kind="ExternalInput")
    wqcT = nc.dram_tensor("wqcT", [513, 256], BF16, kind="ExternalInput")
    wkcT = nc.dram_tensor("wkcT", [513, 256], BF16, kind="ExternalInput")
    wvT = nc.dram_tensor("wvT", [513, 256], BF16, kind="ExternalInput")
    permD = nc.dram_tensor("perm", [128, 128], F16, kind="ExternalInput")
    woT = nc.dram_tensor("woT", [256, EMBED], BF16, kind="ExternalInput")
    cqD = nc.dram_tensor("cq", [128, L], F16, kind="ExternalInput")
    sqD = nc.dram_tensor("sq", [128, L], F16, kind="ExternalInput")
    ckD = nc.dram_tensor("ck", [128, L], F16, kind="ExternalInput")
    skD = nc.dram_tensor("sk", [128, L], F16, kind="ExternalInput")
    triD = nc.dram_tensor("tri", [128, 128], F16, kind="ExternalInput")
    maskD = None
    if use_mask:
        maskD = nc.dram_tensor("maskT", [L, L], F32, kind="ExternalInput")
    outp = nc.dram_tensor("outp", [L, EMBED], F16, kind="ExternalOutput")

    xin = {"q": xqT, "k": xkT, "v": xvT}
    win = {"qc": wqcT, "kc": wkcT, "v": wvT}
    tabin = {"cq": cqD, "sq": sqD, "ck": ckD, "sk": skD}

    with tile.TileContext(nc) as tc:
        with contextlib.ExitStack() as ctx:
            consts = ctx.enter_context(tc.tile_pool(name="consts", bufs=1))
            xpool = ctx.enter_context(tc.tile_pool(name="xpool", bufs=1))
            wpool = ctx.enter_context(tc.tile_pool(name="wpool", bufs=1))
            qkpool = ctx.enter_context(tc.tile_pool(name="qkpool", bufs=1))
            tabpool = ctx.enter_context(tc.tile_pool(name="tabpool", bufs=1))
            vpool = ctx.enter_context(tc.tile_pool(name="vpool", bufs=NS + 4))
            tmp = ctx.enter_context(tc.tile_pool(name="tmp", bufs=3))
            ptpool = ctx.enter_context(tc.tile_pool(name="ptpool", bufs=10))
            npool = ctx.enter_context(tc.tile_pool(name="npool", bufs=3))
            opool = ctx.enter_context(tc.tile_pool(name="opool", bufs=2))
            drpool = ctx.enter_context(
                tc.tile_pool(name="drpool", bufs=3, space="DRAM"))
            mpool = None
            if use_mask:
                mpool = ctx.enter_context(tc.tile_pool(name="mpool", bufs=NS + 2))
            ps_s = ctx.enter_context(tc.tile_pool(name="ps_s", bufs=2, space="PSUM"))
            ps_sm = ctx.enter_context(tc.tile_pool(name="ps_sm", bufs=2, space="PSUM"))
            ps_pv = ctx.enter_context(tc.tile_pool(name="ps_pv", bufs=2, space="PSUM"))

            def body(prev_tail=None):
                # ---- stage-0 small constants ----
                ones_sb = None
                if has_bias:
                    ones_sb = consts.tile([1, L], F16, tag="ones")
                    nc.sync.dma_start(ones_sb[:], xqT[512:513, :])
                tri_sb = consts.tile([128, 128], F16, tag="tri", bufs=2)
                if causal:
                    nc.sync.dma_start(tri_sb[:], triD[:])
                perm_sb = consts.tile([128, 128], F16, tag="perm", bufs=2)
                nc.sync.dma_start(perm_sb[:], permD[:])

                w_sb = {}

                def load_w(nm):
                    wt = wpool.tile([128, 1024], BF16, tag=f"w{nm}",
                                    name=f"w{nm}")
                    wd = win[nm]
                    nc.sync.dma_start(
                        wt[:], bass.AP(tensor=wd[:].tensor, offset=wd[:].offset,
                                       ap=[[256, 128], [32768, 4], [1, 256]]))
                    chunks = [wt[:, c * 256:(c + 1) * 256] for c in range(4)]
                    bt = None
                    if has_bias:
                        bt = wpool.tile([1, 256], BF16, tag=f"w{nm}b",
                                        name=f"w{nm}b")
                        nc.sync.dma_start(bt[:], win[nm][512:513, :])
                    w_sb[nm] = (chunks, bt)

                attnT = [consts.tile([128, L], F16, tag=f"attnT{c}",
                                     name=f"attnT{c}", bufs=2) for c in range(2)]

                qTt = [[None] * NSTRIP for _ in range(2)]  # [e][tb]
                kTt = [[None] * NSTRIP for _ in range(2)]
                vaug = [None] * NS

                xfull = {}

                def load_x_part(nm, lo, hi):
                    if nm not in xfull:
                        xfull[nm] = [xpool.tile([128, L], F16, tag=f"x{nm}{c}",
                                                name=f"x{nm}{c}")
                                     for c in range(4)]
                    for c in range(4):
                        nc.sync.dma_start(
                            xfull[nm][c][:, lo:hi],
                            xin[nm][c * 128:(c + 1) * 128, lo:hi])

                def load_x_tb(nm, tb):
                    return [t[:, tb * TB:(tb + 1) * TB] for t in xfull[nm]]

                tabfull = {}

                def load_tab_part(nm, lo, hi):
                    if nm not in tabfull:
                        tabfull[nm] = tabpool.tile([128, L], F16, tag=nm,
                                                   name=f"{nm}full")
                    nc.sync.dma_start(tabfull[nm][:, lo:hi],
                                      tabin[nm][:, lo:hi])

                def load_tab(nm, tb):
                    return tabfull[nm][:, tb * TB:(tb + 1) * TB]

                def proj_qk(nm, tb, xs, ctab, stab, dst, es_list=(0, 1)):
                    wc, wcb = w_sb[nm + "c"]
                    on = (ones_sb[:, tb * TB:(tb + 1) * TB]
                          if has_bias else None)
                    for e in es_list:
                        es = slice(e * 128, (e + 1) * 128)
                        psc = ps_sm.tile([128, TB], F32, tag="sm",
                                         name=f"ps_{nm}c{e}_{tb}")
                        for c in range(4):
                            nc.tensor.matmul(psc[:], wc[c][:, es], xs[c],
                                             start=(c == 0),
                                             stop=(c == 3 and not has_bias))
                        if has_bias:
                            nc.tensor.matmul(psc[:], wcb[:, es], on,
                                             start=False, stop=True)
                        csb = tmp.tile([128, TB], F16, tag="csb",
                                       name=f"csb{nm}{e}{tb}")
                        nc.vector.tensor_copy(csb[:], psc[:])
                        # rotate-half = row swap r <-> r^32, one perm matmul
                        pss = ps_sm.tile([128, TB], F32, tag="sm",
                                         name=f"ps_{nm}s{e}_{tb}")
                        nc.tensor.matmul(pss[:], perm_sb[:], csb[:],
                                         start=True, stop=True)
                        t1 = tmp.tile([128, TB], F16, tag="t1",
                                      name=f"t1{nm}{e}{tb}")
                        nc.vector.tensor_mul(t1[:], csb[:], ctab)
                        t2 = tmp.tile([128, TB], F16, tag="t2",
                                      name=f"t2{nm}{e}{tb}")
                        nc.vector.tensor_mul(t2[:], pss[:], stab)
                        ot = qkpool.tile([128, TB], F16, tag=f"{nm}T{e}_{tb}",
                                         name=f"{nm}T{e}_{tb}")
                        nc.gpsimd.tensor_add(ot[:], t1[:], t2[:])
                        dst[e][tb] = ot

                def proj_v(tb, xs):
                    wv, wvb = w_sb["v"]
                    for j in range(4):
                        si = tb * 4 + j
                        js = slice(j * 128, (j + 1) * 128)
                        ps = ps_sm.tile([128, 256], F32, tag="sm",
                                        name=f"ps_v{si}")
                        for c in range(4):
                            nc.tensor.matmul(ps[:], xs[c][:, js], wv[c][:],
                                             start=(c == 0),
                                             stop=(c == 3 and not has_bias))
                        if has_bias:
                            nc.tensor.matmul(
                                ps[:], ones_sb[:, si * 128:(si + 1) * 128],
                                wvb[:], start=False, stop=True)
                        va = vpool.tile([128, VW], BF16, tag="vaug",
                                        name=f"vaug{si}")
                        va3 = va[:, 0:HPC * 65].rearrange("p (h c) -> p h c", c=65)
                        nc.vector.tensor_copy(
                            va3[:, :, 0:64],
                            ps[:].rearrange("p (h d) -> p h d", d=64))
                        nc.vector.memset(va3[:, :, 64:65], 1.0)
                        vaug[si] = va

                def flash_strip(T, fillers=()):
                    nsig = 4 * T + 4 if causal else NS
                    mtiles = None
                    if use_mask:
                        mtiles = []
                        for si in range(nsig):
                            mt = mpool.tile([128, TB], F32, tag="mask",
                                            name=f"m{T}_{si}")
                            nc.sync.dma_start(
                                mt[:], maskD[si * 128:(si + 1) * 128,
                                             T * TB:(T + 1) * TB])
                            mtiles.append(mt)
                    d4 = drpool.tile([4, TB], F32, tag="d4", name=f"d4{T}")
                    for p in range(2):
                        # heads A=2p (rows 0:64) and B=2p+1 (rows 64:128) of
                        # the pair-tile run as concurrent row-tiled matmuls
                        # into the two halves of a shared [128,1024] tile.
                        po = [ps_pv.tile([128, TB], F32, tag="pv",
                                         name=f"po{T}p{p}h{half}")
                              for half in range(2)]
                        pts = []
                        coffs = {}
                        for sig in range(nsig):
                            j = sig - 4 * T
                            coff = j * 128 if (causal and j > 0) else 0
                            coffs[sig] = coff
                            ps2 = ps_s.tile([128, 1024], F32, tag="s",
                                            name=f"S{T}p{p}s{sig}")
                            kt = kTt[p][sig // 4]
                            qt = qTt[p][T]
                            scols = slice((sig % 4) * 128, (sig % 4 + 1) * 128)
                            for half, hb in ((0, 0), (1, 64)):
                                nc.tensor.matmul(
                                    ps2[:, half * TB + coff:(half + 1) * TB],
                                    kt[hb:hb + 64, scols],
                                    qt[hb:hb + 64, coff:TB],
                                    start=True, stop=True,
                                    tile_position=(hb, 0))
                            if use_mask:
                                for half in range(2):
                                    sl = slice(half * TB + coff,
                                               (half + 1) * TB)
                                    nc.vector.tensor_add(
                                        ps2[:, sl], ps2[:, sl],
                                        mtiles[sig][:, coff:TB])
                            pt = ptpool.tile([128, 1024], BF16, tag="pt",
                                             name=f"P{T}p{p}s{sig}")
                            nc.scalar.activation(pt[:, coff:1024],
                                                 ps2[:, coff:1024],
                                                 mybir.ActivationFunctionType.Exp)
                            if causal and j >= 0:
                                pta = pt[:]
                                blk = bass.AP(
                                    tensor=pta.tensor,
                                    offset=pta.offset + j * 128,
                                    ap=[[1024, 128], [TB, 2], [1, 128]])
                                tri_rep = bass.AP(
                                    tensor=tri_sb[:].tensor,
                                    offset=tri_sb[:].offset,
                                    ap=[[128, 128], [0, 2], [1, 128]])
                                nc.vector.tensor_mul(blk, blk, tri_rep)
                            pts.append(pt)
                            for half in range(2):
                                h = 2 * p + half
                                nc.tensor.matmul(
                                    po[half][:, coff:TB],
                                    vaug[sig][:, h * 65:h * 65 + 128],
                                    pt[:, half * TB + coff:(half + 1) * TB],
                                    start=(sig == 0), stop=(sig == nsig - 1))
                        # normalization: copy out of PSUM, pair-batched
                        # reciprocal via a DRAM [64,16] reshape, broadcast,
                        # Pool-engine multiply into attnT (odd heads need a
                        # partition-shift DMA via a staging tile).
                        pocs = []
                        for half in range(2):
                            h = 2 * p + half
                            poc = npool.tile([65, TB], F32, tag="poc",
                                             name=f"poc{T}h{h}", bufs=6)
                            nc.vector.tensor_copy(poc[:], po[half][0:65, :])
                            nc.sync.dma_start(d4[h:h + 1, :], poc[64:65, :])
                            pocs.append(poc)
                        rsb = npool.tile([64, 16], F32, tag="rsb",
                                         name=f"rsb{T}p{p}", bufs=4)
                        nc.sync.dma_start(
                            rsb[:], bass.AP(tensor=d4[:].tensor,
                                            offset=d4[:].offset + p * 2 * TB,
                                            ap=[[16, 64], [1, 16]]))
                        rrec = npool.tile([64, 16], F32, tag="rrec",
                                          name=f"rrec{T}p{p}", bufs=4)
                        nc.vector.reciprocal(rrec[:], rsb[:])
                        dr2 = drpool.tile([2, TB], F32, tag="dr2",
                                          name=f"dr2{T}p{p}")
                        nc.sync.dma_start(
                            bass.AP(tensor=dr2[:].tensor, offset=dr2[:].offset,
                                    ap=[[16, 64], [1, 16]]), rrec[:])
                        for half in range(2):
                            rcp = npool.tile([64, TB], F32, tag="rcp",
                                             name=f"rcp{T}p{p}h{half}", bufs=6)
                            bcast = bass.AP(tensor=dr2[:].tensor,
                                            offset=dr2[:].offset + half * TB,
                                            ap=[[0, 64], [1, TB]])
                            nc.sync.dma_start(rcp[:], bcast)
                            strip_norms.setdefault(T, []).append(
                                (pocs[half], rcp, p, half, T))
                        if p < len(fillers) and fillers[p] is not None:
                            fillers[p]()
                    for f in fillers[2:]:
                        if f is not None:
                            f()

                def drain_norms(T, eng=None):
                    eng = eng or nc.gpsimd
                    for poc, rcp, ht, odd, _T in strip_norms.pop(T, []):
                        tcols = slice(_T * TB, (_T + 1) * TB)
                        if not odd:
                            eng.tensor_mul(attnT[ht][0:64, tcols],
                                           poc[0:64, :], rcp[:])
                        else:
                            stag = npool.tile([64, TB], F16, tag="stag",
                                              name=f"stag{_T}x{ht}", bufs=4)
                            eng.tensor_mul(stag[:], poc[0:64, :], rcp[:])
                            nc.sync.dma_start(attnT[ht][64:128, tcols], stag[:])

                def out_proj(taus):
                    taus = list(taus)
                    osb = opool.tile([128, 4 * EMBED], F16, tag="osb",
                                     name=f"osb{taus[0]}")
                    for i, tau in enumerate(taus):
                        ps = ps_sm.tile([128, EMBED], F32, tag="sm",
                                         name=f"ps_o{tau}")
                        for c in range(2):
                            nc.tensor.matmul(
                                ps[:], attnT[c][:, tau * 128:(tau + 1) * 128],
                                wo_sb[c], start=(c == 0), stop=(c == 1))
                        nc.vector.tensor_copy(
                            osb[:, i * EMBED:(i + 1) * EMBED], ps[:])
                    nc.sync.dma_start(
                        bass.AP(tensor=outp[:].tensor,
                                offset=taus[0] * 128 * EMBED,
                                ap=[[EMBED, 128], [128 * EMBED, 4], [1, EMBED]]),
                        osb[:])

                strip_norms = {}

                # --- prologue: strip-0 data prioritized, then remainders ---
                load_w("qc")
                load_x_part("q", 0, TB)
                load_tab_part("cq", 0, TB)
                load_tab_part("sq", 0, TB)
                proj_qk("q", 0, load_x_tb("q", 0), load_tab("cq", 0),
                        load_tab("sq", 0), qTt, es_list=(0,))
                load_w("kc")
                load_x_part("k", 0, TB)
                load_tab_part("ck", 0, TB)
                load_tab_part("sk", 0, TB)
                proj_qk("k", 0, load_x_tb("k", 0), load_tab("ck", 0),
                        load_tab("sk", 0), kTt, es_list=(0,))
                load_w("v")
                load_x_part("v", 0, TB)
                proj_v(0, load_x_tb("v", 0))
                proj_qk("q", 0, load_x_tb("q", 0), load_tab("cq", 0),
                        load_tab("sq", 0), qTt, es_list=(1,))
                proj_qk("k", 0, load_x_tb("k", 0), load_tab("ck", 0),
                        load_tab("sk", 0), kTt, es_list=(1,))
                load_x_part("q", TB, L)
                load_tab_part("cq", TB, L)
                load_tab_part("sq", TB, L)
                load_x_part("k", TB, L)
                load_tab_part("ck", TB, L)
                load_tab_part("sk", TB, L)
                load_x_part("v", TB, L)
                wot = consts.tile([128, 2 * EMBED], BF16, tag="wo",
                                  name="wo", bufs=2)
                nc.sync.dma_start(
                    wot[:], bass.AP(tensor=woT[:].tensor, offset=woT[:].offset,
                                    ap=[[512, 128], [65536, 2], [1, 512]]))
                wo_sb = [wot[:, c * EMBED:(c + 1) * EMBED] for c in range(2)]

                def mk_proj(nm, tb):
                    def f():
                        proj_qk(nm, tb, load_x_tb(nm, tb),
                                load_tab("c" + nm, tb), load_tab("s" + nm, tb),
                                qTt if nm == "q" else kTt)
                    return f

                def mk_projv(tb):
                    return lambda: proj_v(tb, load_x_tb("v", tb))

                def mk_drain(tb, eng=None):
                    return lambda: drain_norms(tb, eng)

                def mk_out(tb):
                    return lambda: out_proj(range(tb * 4, (tb + 1) * 4))

                for tb in range(NSTRIP):
                    fill = []
                    if tb + 1 < NSTRIP:
                        fill.append(mk_proj("q", tb + 1))
                    if tb == 0 and prev_tail is not None:
                        fill.append(prev_tail[0])   # prev body's drain
                    if tb >= 1:
                        fill.append(mk_drain(tb - 1))
                    if tb + 1 < NSTRIP:
                        fill += [mk_proj("k", tb + 1), mk_projv(tb + 1)]
                    if tb == 0 and prev_tail is not None:
                        fill.append(prev_tail[1])   # prev body's out_proj: late
                    if tb >= 2:
                        fill.append(mk_out(tb - 2))
                    flash_strip(tb, fill)
                out_proj(range((NSTRIP - 2) * 4, (NSTRIP - 1) * 4))
                return (mk_drain(NSTRIP - 1), mk_out(NSTRIP - 1))

            if reps > 1 and reps <= 4:
                # straight-line repetition (for TimelineSim marginal analysis)
                tail = None
                for _ in range(reps):
                    tail = body(tail)
                tail[0]()
                tail[1]()
            elif reps > 1:
                unroll = 4 if reps % 4 == 0 else 1
                with tc.For_i(0, reps // unroll, 1,
                              staggered_reset=True,
                              hint_engines=(mybir.EngineType.PE,
                                            mybir.EngineType.Activation,
                                            mybir.EngineType.DVE,
                                            mybir.EngineType.SP,
                                            mybir.EngineType.Pool)):
                    tail = None
                    for _ in range(unroll):
                        tail = body(tail)
                    tail[0]()
                    tail[1]()
            else:
                t0_, t1_ = body()
                t0_()
                t1_()

    nc.compile()
    return nc


_PROGRAM_CACHE = {}


def get_program(causal: bool, use_mask: bool, has_bias: bool, reps: int = 1):
    key = (causal, use_mask, has_bias, reps)
    if key not in _PROGRAM_CACHE:
        _PROGRAM_CACHE[key] = _build_program(causal, use_mask, has_bias, reps)
    return _PROGRAM_CACHE[key]


def _prep_in_maps(query, key, value, key_padding_mask, attn_mask,
                  Wq, bq, Wk, bk, Wv, bv, Wo, bo, use_mask, has_bias):
    """Build the 8 per-core input dicts."""
    import ml_dtypes
    cq, sq, ck, sk = _xpos_tables()
    tri = np.where(np.arange(128)[None, :] >= np.arange(128)[:, None],
                   np.float16(1.0), np.float16(0.0)).astype(np.float16)
    # rotate-half permutation: out row r = in row r^32 (symmetric involution)
    perm = np.zeros((128, 128), np.float16)
    perm[np.arange(128) ^ 32, np.arange(128)] = np.float16(1.0)

    def aug_x(x):
        a = np.empty((513, L), np.float16)
        a[0:512] = np.asarray(x, np.float32).T.astype(np.float16)
        a[512] = np.float16(1.0)
        return a

    xqTs = [aug_x(query[b]) for b in range(B)]
    xkTs = [aug_x(key[b]) for b in range(B)]
    xvTs = [aug_x(value[b]) for b in range(B)]

    masks = None
    if use_mask:
        am = np.asarray(attn_mask, np.float32)
        kp = np.asarray(key_padding_mask)
        masks = []
        for b in range(B):
            m = am.copy()
            if kp[b].any():
                m = m + np.where(kp[b], np.float32(-1e30),
                                 np.float32(0.0))[None, :]
            masks.append(np.ascontiguousarray(m.T.astype(np.float32)))

    Wq = np.asarray(Wq, np.float32); bq = np.asarray(bq, np.float32)
    Wk = np.asarray(Wk, np.float32); bk = np.asarray(bk, np.float32)
    Wv = np.asarray(Wv, np.float32); bv = np.asarray(bv, np.float32)
    Wo = np.asarray(Wo, np.float32)

    in_maps = []
    for core in range(N_CORES):
        b, hg = core // 2, core % 2
        hs = hg * HPC
        idx_p = np.concatenate(
            [hs * HD + hl * HD + _PERM64 for hl in range(HPC)])
        # sin-projection rows: within each head's 64-block, row r <- r XOR 32
        xor = (np.arange(256).reshape(HPC, HD)[:, (np.arange(HD) ^ 32)]
               ).reshape(-1)
        idx_s = idx_p[xor]
        idx_v = hs * HD + np.arange(HPC * HD)

        def aug_w(W, bias, idx):
            a = np.empty((513, 256), np.float32)
            a[0:512] = np.ascontiguousarray(W[idx, :].T)
            a[512] = bias[idx]
            return a.astype(ml_dtypes.bfloat16)

        m = {
            "xqT": xqTs[b], "xkT": xkTs[b], "xvT": xvTs[b],
            "wqcT": aug_w(Wq, bq, idx_p),
            "wkcT": aug_w(Wk, bk, idx_p),
            "wvT": aug_w(Wv, bv, idx_v),
            "woT": np.ascontiguousarray(Wo[:, idx_v].T).astype(ml_dtypes.bfloat16),
            "cq": cq, "sq": sq, "ck": ck, "sk": sk,
            "tri": tri, "perm": perm,
        }
        if use_mask:
            m["maskT"] = masks[b]
        in_maps.append(m)
    return in_maps


def classify_mask(attn_mask, key_padding_mask):
    am = np.asarray(attn_mask, np.float32)
    kp = np.asarray(key_padding_mask)
    if not kp.any():
        causal = np.where(
            np.tril(np.ones((L, L), bool)), np.float32(0.0),
            np.float32(NEG)).astype(np.float32)
        if np.array_equal(am, causal):
            return True, False
        if not am.any():
            return False, False
    return False, True


def kernel(query, key, value, key_padding_mask, attn_mask,
           Wq, bq, Wk, bk, Wv, bv, Wo, bo):
    causal, use_mask = classify_mask(attn_mask, key_padding_mask)
    has_bias = bool(np.asarray(bq).any() or np.asarray(bk).any()
                    or np.asarray(bv).any())
    nc = get_program(causal, use_mask, has_bias, reps=1)
    in_maps = _prep_in_maps(query, key, value, key_padding_mask, attn_mask,
                            Wq, bq, Wk, bk, Wv, bv, Wo, bo, use_mask, has_bias)
    res = run_bass_kernel_spmd(nc, in_maps, list(range(N_CORES)))
    bo = np.asarray(bo, np.float32)
    out = np.empty((B, L, EMBED), np.float32)
    for b in range(B):
        out[b] = (res.results[2 * b]["outp"].astype(np.float32)
                  + res.results[2 * b + 1]["outp"].astype(np.float32)
                  + bo[None, :])
    return out


# revision 45
# speedup vs baseline: 3.0505x; 1.0108x over previous
"""Bass/Trainium2 kernel for nn_BerpXposMultiHeadedAttention (8-core SPMD).

Sharding: data-parallel over batch (4 batches x 2 cores) x tensor-parallel over
heads (4 heads per core).  Each core computes its 4 heads of flash-style xpos
attention for its batch plus the row-sharded partial out-projection; the host
sums the two partials per batch (the "all-reduce") and adds the output bias.

Design notes (measured on HW via reps-slope + TimelineSim occupancy):
- All matmul operands are 16-bit: bf16 weights (stationary) x fp16 activations,
  fp16 q/k for QK^T (enables fast weight loads), bf16 probabilities for P@V.
  fp32 PSUM accumulation keeps softmax inputs accurate (rel err ~2.7e-3).
- Head-PAIRED QK^T: the two heads of a q/k pair-tile run as concurrent
  row-tiled matmuls (tile_position (0,0)/(64,0), K=64 each) writing the two
  halves of a shared [128,1024] score tile -> one exp feeds both heads and
  QK^T costs half the PE time.  Measured ~16us/iter win.
- xpos rotation via a SINGLE projection per q/k: the rotate-half partner is a
  row swap r <-> r^32 in the deinterleaved layout, computed by one matmul
  against a constant 128x128 permutation (saves 3 of 8 proj matmuls and half
  the q/k weight DMA); combine = two DVE muls (fp16 tables) + one Pool add.
- Causal fast path trims score matmuls, exp, and P@V moving columns to the
  exact 128-block diagonal; the diagonal block is masked by a post-exp 0/1
  lower-triangle multiply on bf16 P (off the PSUM critical path).
- Softmax: P@V accumulates a ones-row per head (PSUM row 64 = sumexp);
  per-pair denominators go through a DRAM [64,16]-reshape so one 6-cpe DVE
  reciprocal covers 1024 values in ~100ns, then a stride-0 DMA broadcast and
  a Pool multiply normalize into attnT (odd heads partition-shift via DMA).
- PSUM: scores double-buffered (2x2 banks), 1-bank slots for proj/v/out (2),
  2 po banks = 8.  Single-DMA weight loads ([513,256] -> [128,1024] tiles);
  out-projection staged per strip into one [128,2048] tile -> one DMA.
- Emission: strip-0 loads prioritized; proj of strip T+1, norm-drain and
  out-proj of strip T-1 are emitted as fillers inside flash strip T; each
  body passes its final drain+out_proj into the NEXT body (engines are
  in-order, so tail work must sit behind the next body head in the stream).
- Timing loop: For_i over reps/4 with 4 unrolled bodies and staggered_reset
  (the plain loop ends in an all-engine barrier which serializes iterations).
"""

import sys

sys.path.insert(0, "/opt/trn_rl_repo")

import contextlib

import numpy as np

import concourse.bacc as bacc
import concourse.bass as bass
import concourse.tile as tile
from concourse import mybir
from concourse.bass_utils import run_bass_kernel_spmd

# Problem constants (hardcoded per the task contract).
B = 4
L = 2048
EMBED = 512
HEADS = 8
HD = 64
SCALE_BASE = 512
NEG = -1e9
N_CORES = 8
HPC = 4           # heads per core
TB = 512          # t-block (strip) width
NT = L // 128     # 16 t-chunks
NS = L // 128     # 16 s-chunks
NSTRIP = L // TB  # 4 strips
VW = 328          # v_aug tile width (4 heads x 65 + 68 junk tail)

F32 = mybir.dt.float32
F32R = mybir.dt.float32r
F16 = mybir.dt.float16
BF16 = mybir.dt.bfloat16

# Deinterleave permutation of a 64-wide head dim: evens then odds.
_PERM64 = np.concatenate([np.arange(0, HD, 2), np.arange(1, HD, 2)])


def _xpos_tables():
    """Host-side xpos cos/sin tables in the permuted [d, t] layout.

    Returns (cq, sq, ck, sk), each [128, L] float16 (two heads' worth of rows,
    identical per head).  The 1/sqrt(HD) score scale is folded into the q pair.
    """
    d = HD
    base = ((np.arange(0, d, 2, dtype=np.float32) + np.float32(0.4 * d))
            / np.float32(1.4 * d)).astype(np.float32)                    # [32]
    min_pos = -(L // 2)
    power = (np.arange(min_pos, L + min_pos, dtype=np.float32)
             / np.float32(SCALE_BASE))                                   # [L]
    scale = (base[None, :] ** power[:, None]).astype(np.float32)         # [L, 32]
    half = d // 2
    inv_freq = (1.0 / (10000.0 ** (np.arange(half, dtype=np.float32) / half))
                ).astype(np.float32)
    sinusoid = np.arange(L, dtype=np.float32)[:, None] * inv_freq[None, :]
    sin = np.sin(sinusoid).astype(np.float32)
    cos = np.cos(sinusoid).astype(np.float32)

    def pack(cs, ss, fold):
        cs = (cs * fold).astype(np.float32)
        ss = (ss * fold).astype(np.float32)
        # permuted layout: rows 0:32 <- even orig dims, rows 32:64 <- odd.
        cos_p = np.concatenate([cs.T, cs.T], axis=0)      # [64, L]
        sin_p = np.concatenate([-ss.T, ss.T], axis=0)     # [64, L]
        return (np.concatenate([cos_p, cos_p], axis=0).astype(np.float16),
                np.concatenate([sin_p, sin_p], axis=0).astype(np.float16))

    inv_scale = (1.0 / scale).astype(np.float32)
    cq, sq = pack(cos * scale, sin * scale, np.float32(HD ** -0.5))
    ck, sk = pack(cos * inv_scale, sin * inv_scale, np.float32(1.0))
    return cq, sq, ck, sk


def _build_program(causal: bool, use_mask: bool, has_bias: bool, reps: int = 1):
    nc = bacc.Bacc("TRN2", target_bir_lowering=False, debug=False,
                   num_devices=N_CORES)

    # ---- DRAM I/O -------------------------------------------------------
    xqT = nc.dram_tensor("xqT", [513, L], F16, kind="ExternalInput")
    xkT = nc.dram_tensor("xkT", [513, L], F16, kind="ExternalInput")
    xvT = nc.dram_tensor("xvT", [513, L], F16, kind="ExternalInput")
    wqcT = nc.dram_tensor("wqcT", [513, 256], BF16, kind="ExternalInput")
    wkcT = nc.dram_tensor("wkcT", [513, 256], BF16, kind="ExternalInput")
    wvT = nc.dram_tensor("wvT", [513, 256], BF16, kind="ExternalInput")
    permD = nc.dram_tensor("perm", [128, 128], F16, kind="ExternalInput")
    woT = nc.dram_tensor("woT", [256, EMBED], BF16, kind="ExternalInput")
    cqD = nc.dram_tensor("cq", [128, L], F16, kind="ExternalInput")
    sqD = nc.dram_tensor("sq", [128, L], F16, kind="ExternalInput")
    ckD = nc.dram_tensor("ck", [128, L], F16, kind="ExternalInput")
    skD = nc.dram_tensor("sk", [128, L], F16, kind="ExternalInput")
    triD = nc.dram_tensor("tri", [128, 128], F16, kind="ExternalInput")
    maskD = None
    if use_mask:
        maskD = nc.dram_tensor("maskT", [L, L], F32, kind="ExternalInput")
    outp = nc.dram_tensor("outp", [L, EMBED], F16, kind="ExternalOutput")

    xin = {"q": xqT, "k": xkT, "v": xvT}
    win = {"qc": wqcT, "kc": wkcT, "v": wvT}
    tabin = {"cq": cqD, "sq": sqD, "ck": ckD, "sk": skD}

    with tile.TileContext(nc) as tc:
        with contextlib.ExitStack() as ctx:
            consts = ctx.enter_context(tc.tile_pool(name="consts", bufs=1))
            xpool = ctx.enter_context(tc.tile_pool(name="xpool", bufs=1))
            wpool = ctx.enter_context(tc.tile_pool(name="wpool", bufs=1))
            qkpool = ctx.enter_context(tc.tile_pool(name="qkpool", bufs=2))
            tabpool = ctx.enter_context(tc.tile_pool(name="tabpool", bufs=1))
            vpool = ctx.enter_context(tc.tile_pool(name="vpool", bufs=NS + 4))
            tmp = ctx.enter_context(tc.tile_pool(name="tmp", bufs=3))
            ptpool = ctx.enter_context(tc.tile_pool(name="ptpool", bufs=10))
            npool = ctx.enter_context(tc.tile_pool(name="npool", bufs=3))
            opool = ctx.enter_context(tc.tile_pool(name="opool", bufs=2))
            drpool = ctx.enter_context(
                tc.tile_pool(name="drpool", bufs=3, space="DRAM"))
            mpool = None
            if use_mask:
                mpool = ctx.enter_context(tc.tile_pool(name="mpool", bufs=NS + 2))
            ps_s = ctx.enter_context(tc.tile_pool(name="ps_s", bufs=2, space="PSUM"))
            ps_sm = ctx.enter_context(tc.tile_pool(name="ps_sm", bufs=2, space="PSUM"))
            ps_pv = ctx.enter_context(tc.tile_pool(name="ps_pv", bufs=2, space="PSUM"))

            def body(prev_tail=None):
                # ---- stage-0 small constants ----
                ones_sb = None
                if has_bias:
                    ones_sb = consts.tile([1, L], F16, tag="ones")
                    nc.sync.dma_start(ones_sb[:], xqT[512:513, :])
                tri_sb = consts.tile([128, 128], F16, tag="tri", bufs=2)
                if causal:
                    nc.sync.dma_start(tri_sb[:], triD[:])
                perm_sb = consts.tile([128, 128], F16, tag="perm", bufs=2)
                nc.sync.dma_start(perm_sb[:], permD[:])

                w_sb = {}

                def load_w(nm):
                    wt = wpool.tile([128, 1024], BF16, tag=f"w{nm}",
                                    name=f"w{nm}")
                    wd = win[nm]
                    nc.sync.dma_start(
                        wt[:], bass.AP(tensor=wd[:].tensor, offset=wd[:].offset,
                                       ap=[[256, 128], [32768, 4], [1, 256]]))
                    chunks = [wt[:, c * 256:(c + 1) * 256] for c in range(4)]
                    bt = None
                    if has_bias:
                        bt = wpool.tile([1, 256], BF16, tag=f"w{nm}b",
                                        name=f"w{nm}b")
                        nc.sync.dma_start(bt[:], win[nm][512:513, :])
                    w_sb[nm] = (chunks, bt)

                attnT = [consts.tile([128, L], F16, tag=f"attnT{c}",
                                     name=f"attnT{c}", bufs=2) for c in range(2)]

                qTt = [[None] * NSTRIP for _ in range(2)]  # [e][tb]
                kTt = [[None] * NSTRIP for _ in range(2)]
                vaug = [None] * NS

                xfull = {}

                def load_x_part(nm, lo, hi):
                    if nm not in xfull:
                        xfull[nm] = [xpool.tile([128, L], F16, tag=f"x{nm}{c}",
                                                name=f"x{nm}{c}")
                                     for c in range(4)]
                    for c in range(4):
                        nc.sync.dma_start(
                            xfull[nm][c][:, lo:hi],
                            xin[nm][c * 128:(c + 1) * 128, lo:hi])

                def load_x_tb(nm, tb):
                    return [t[:, tb * TB:(tb + 1) * TB] for t in xfull[nm]]

                tabfull = {}

                def load_tab_part(nm, lo, hi):
                    if nm not in tabfull:
                        tabfull[nm] = tabpool.tile([128, L], F16, tag=nm,
                                                   name=f"{nm}full")
                    nc.sync.dma_start(tabfull[nm][:, lo:hi],
                                      tabin[nm][:, lo:hi])

                def load_tab(nm, tb):
                    return tabfull[nm][:, tb * TB:(tb + 1) * TB]

                def proj_qk(nm, tb, xs, ctab, stab, dst, es_list=(0, 1)):
                    wc, wcb = w_sb[nm + "c"]
                    on = (ones_sb[:, tb * TB:(tb + 1) * TB]
                          if has_bias else None)
                    for e in es_list:
                        es = slice(e * 128, (e + 1) * 128)
                        psc = ps_sm.tile([128, TB], F32, tag="sm",
                                         name=f"ps_{nm}c{e}_{tb}")
                        for c in range(4):
                            nc.tensor.matmul(psc[:], wc[c][:, es], xs[c],
                                             start=(c == 0),
                                             stop=(c == 3 and not has_bias))
                        if has_bias:
                            nc.tensor.matmul(psc[:], wcb[:, es], on,
                                             start=False, stop=True)
                        csb = tmp.tile([128, TB], F16, tag="csb",
                                       name=f"csb{nm}{e}{tb}")
                        nc.vector.tensor_copy(csb[:], psc[:])
                        # rotate-half = row swap r <-> r^32, one perm matmul
                        pss = ps_sm.tile([128, TB], F32, tag="sm",
                                         name=f"ps_{nm}s{e}_{tb}")
                        nc.tensor.matmul(pss[:], perm_sb[:], csb[:],
                                         start=True, stop=True)
                        t1 = tmp.tile([128, TB], F16, tag="t1",
                                      name=f"t1{nm}{e}{tb}")
                        nc.vector.tensor_mul(t1[:], csb[:], ctab)
                        t2 = tmp.tile([128, TB], F16, tag="t2",
                                      name=f"t2{nm}{e}{tb}")
                        nc.vector.tensor_mul(t2[:], pss[:], stab)
                        ot = qkpool.tile([128, TB], F16, tag=f"{nm}T{e}_{tb}",
                                         name=f"{nm}T{e}_{tb}")
                        nc.gpsimd.tensor_add(ot[:], t1[:], t2[:])
                        dst[e][tb] = ot

                def proj_v(tb, xs):
                    wv, wvb = w_sb["v"]
                    for j in range(4):
                        si = tb * 4 + j
                        js = slice(j * 128, (j + 1) * 128)
                        ps = ps_sm.tile([128, 256], F32, tag="sm",
                                        name=f"ps_v{si}")
                        for c in range(4):
                            nc.tensor.matmul(ps[:], xs[c][:, js], wv[c][:],
                                             start=(c == 0),
                                             stop=(c == 3 and not has_bias))
                        if has_bias:
                            nc.tensor.matmul(
                                ps[:], ones_sb[:, si * 128:(si + 1) * 128],
                                wvb[:], start=False, stop=True)
                        va = vpool.tile([128, VW], BF16, tag="vaug",
                                        name=f"vaug{si}")
                        va3 = va[:, 0:HPC * 65].rearrange("p (h c) -> p h c", c=65)
                        nc.vector.tensor_copy(
                            va3[:, :, 0:64],
                            ps[:].rearrange("p (h d) -> p h d", d=64))
                        nc.vector.memset(va3[:, :, 64:65], 1.0)
                        vaug[si] = va

                def flash_strip(T, fillers=()):
                    nsig = 4 * T + 4 if causal else NS
                    mtiles = None
                    if use_mask:
                        mtiles = []
                        for si in range(nsig):
                            mt = mpool.tile([128, TB], F32, tag="mask",
                                            name=f"m{T}_{si}")
                            nc.sync.dma_start(
                                mt[:], maskD[si * 128:(si + 1) * 128,
                                             T * TB:(T + 1) * TB])
                            mtiles.append(mt)
                    d4 = drpool.tile([4, TB], F32, tag="d4", name=f"d4{T}")
                    for p in range(2):
                        # heads A=2p (rows 0:64) and B=2p+1 (rows 64:128) of
                        # the pair-tile run as concurrent row-tiled matmuls
                        # into the two halves of a shared [128,1024] tile.
                        po = [ps_pv.tile([128, TB], F32, tag="pv",
                                         name=f"po{T}p{p}h{half}")
                              for half in range(2)]
                        pts = []
                        coffs = {}
                        for sig in range(nsig):
                            j = sig - 4 * T
                            coff = j * 128 if (causal and j > 0) else 0
                            coffs[sig] = coff
                            ps2 = ps_s.tile([128, 1024], F32, tag="s",
                                            name=f"S{T}p{p}s{sig}")
                            kt = kTt[p][sig // 4]
                            qt = qTt[p][T]
                            scols = slice((sig % 4) * 128, (sig % 4 + 1) * 128)
                            for half, hb in ((0, 0), (1, 64)):
                                nc.tensor.matmul(
                                    ps2[:, half * TB + coff:(half + 1) * TB],
                                    kt[hb:hb + 64, scols],
                                    qt[hb:hb + 64, coff:TB],
                                    start=True, stop=True,
                                    tile_position=(hb, 0))
                            if use_mask:
                                for half in range(2):
                                    sl = slice(half * TB + coff,
                                               (half + 1) * TB)
                                    nc.vector.tensor_add(
                                        ps2[:, sl], ps2[:, sl],
                                        mtiles[sig][:, coff:TB])
                            pt = ptpool.tile([128, 1024], BF16, tag="pt",
                                             name=f"P{T}p{p}s{sig}")
                            nc.scalar.activation(pt[:, coff:1024],
                                                 ps2[:, coff:1024],
                                                 mybir.ActivationFunctionType.Exp)
                            if causal and j >= 0:
                                pta = pt[:]
                                blk = bass.AP(
                                    tensor=pta.tensor,
                                    offset=pta.offset + j * 128,
                                    ap=[[1024, 128], [TB, 2], [1, 128]])
                                tri_rep = bass.AP(
                                    tensor=tri_sb[:].tensor,
                                    offset=tri_sb[:].offset,
                                    ap=[[128, 128], [0, 2], [1, 128]])
                                nc.vector.tensor_mul(blk, blk, tri_rep)
                            pts.append(pt)
                            for half in range(2):
                                h = 2 * p + half
                                nc.tensor.matmul(
                                    po[half][:, coff:TB],
                                    vaug[sig][:, h * 65:h * 65 + 128],
                                    pt[:, half * TB + coff:(half + 1) * TB],
                                    start=(sig == 0), stop=(sig == nsig - 1))
                        # normalization: copy out of PSUM, pair-batched
                        # reciprocal via a DRAM [64,16] reshape, broadcast,
                        # Pool-engine multiply into attnT (odd heads need a
                        # partition-shift DMA via a staging tile).
                        pocs = []
                        for half in range(2):
                            h = 2 * p + half
                            poc = npool.tile([65, TB], F32, tag="poc",
                                             name=f"poc{T}h{h}", bufs=6)
                            nc.vector.tensor_copy(poc[:], po[half][0:65, :])
                            nc.sync.dma_start(d4[h:h + 1, :], poc[64:65, :])
                            pocs.append(poc)
                        rsb = npool.tile([64, 16], F32, tag="rsb",
                                         name=f"rsb{T}p{p}", bufs=4)
                        nc.sync.dma_start(
                            rsb[:], bass.AP(tensor=d4[:].tensor,
                                            offset=d4[:].offset + p * 2 * TB,
                                            ap=[[16, 64], [1, 16]]))
                        rrec = npool.tile([64, 16], F32, tag="rrec",
                                          name=f"rrec{T}p{p}", bufs=4)
                        nc.vector.reciprocal(rrec[:], rsb[:])
                        dr2 = drpool.tile([2, TB], F32, tag="dr2",
                                          name=f"dr2{T}p{p}")
                        nc.sync.dma_start(
                            bass.AP(tensor=dr2[:].tensor, offset=dr2[:].offset,
                                    ap=[[16, 64], [1, 16]]), rrec[:])
                        for half in range(2):
                            rcp = npool.tile([64, TB], F32, tag="rcp",
                                             name=f"rcp{T}p{p}h{half}", bufs=6)
                            bcast = bass.AP(tensor=dr2[:].tensor,
                                            offset=dr2[:].offset + half * TB,
                                            ap=[[0, 64], [1, TB]])
                            nc.sync.dma_start(rcp[:], bcast)
                            strip_norms.setdefault(T, []).append(
                                (pocs[half], rcp, p, half, T))
                        if p < len(fillers) and fillers[p] is not None:
                            fillers[p]()
                    for f in fillers[2:]:
                        if f is not None:
                            f()

                def drain_norms(T, eng=None):
                    eng = eng or nc.gpsimd
                    for poc, rcp, ht, odd, _T in strip_norms.pop(T, []):
                        tcols = slice(_T * TB, (_T + 1) * TB)
                        if not odd:
                            eng.tensor_mul(attnT[ht][0:64, tcols],
                                           poc[0:64, :], rcp[:])
                        else:
                            stag = npool.tile([64, TB], F16, tag="stag",
                                              name=f"stag{_T}x{ht}", bufs=4)
                            eng.tensor_mul(stag[:], poc[0:64, :], rcp[:])
                            nc.sync.dma_start(attnT[ht][64:128, tcols], stag[:])

                def out_proj(taus):
                    taus = list(taus)
                    osb = opool.tile([128, 4 * EMBED], F16, tag="osb",
                                     name=f"osb{taus[0]}")
                    for i, tau in enumerate(taus):
                        ps = ps_sm.tile([128, EMBED], F32, tag="sm",
                                         name=f"ps_o{tau}")
                        for c in range(2):
                            nc.tensor.matmul(
                                ps[:], attnT[c][:, tau * 128:(tau + 1) * 128],
                                wo_sb[c], start=(c == 0), stop=(c == 1))
                        nc.vector.tensor_copy(
                            osb[:, i * EMBED:(i + 1) * EMBED], ps[:])
                    nc.sync.dma_start(
                        bass.AP(tensor=outp[:].tensor,
                                offset=taus[0] * 128 * EMBED,
                                ap=[[EMBED, 128], [128 * EMBED, 4], [1, EMBED]]),
                        osb[:])

                strip_norms = {}

                # --- prologue: strip-0 data prioritized, then remainders ---
                load_w("qc")
                load_x_part("q", 0, TB)
                load_tab_part("cq", 0, TB)
                load_tab_part("sq", 0, TB)
                proj_qk("q", 0, load_x_tb("q", 0), load_tab("cq", 0),
                        load_tab("sq", 0), qTt, es_list=(0,))
                load_w("kc")
                load_x_part("k", 0, TB)
                load_tab_part("ck", 0, TB)
                load_tab_part("sk", 0, TB)
                proj_qk("k", 0, load_x_tb("k", 0), load_tab("ck", 0),
                        load_tab("sk", 0), kTt, es_list=(0,))
                load_w("v")
                load_x_part("v", 0, TB)
                proj_v(0, load_x_tb("v", 0))
                proj_qk("q", 0, load_x_tb("q", 0), load_tab("cq", 0),
                        load_tab("sq", 0), qTt, es_list=(1,))
                proj_qk("k", 0, load_x_tb("k", 0), load_tab("ck", 0),
                        load_tab("sk", 0), kTt, es_list=(1,))
                load_x_part("q", TB, L)
                load_tab_part("cq", TB, L)
                load_tab_part("sq", TB, L)
                load_x_part("k", TB, L)
                load_tab_part("ck", TB, L)
                load_tab_part("sk", TB, L)
                load_x_part("v", TB, L)
                wot = consts.tile([128, 2 * EMBED], BF16, tag="wo",
                                  name="wo", bufs=2)
                nc.sync.dma_start(
                    wot[:], bass.AP(tensor=woT[:].tensor, offset=woT[:].offset,
                                    ap=[[512, 128], [65536, 2], [1, 512]]))
                wo_sb = [wot[:, c * EMBED:(c + 1) * EMBED] for c in range(2)]

                def mk_proj(nm, tb):
                    def f():
                        proj_qk(nm, tb, load_x_tb(nm, tb),
                                load_tab("c" + nm, tb), load_tab("s" + nm, tb),
                                qTt if nm == "q" else kTt)
                    return f

                def mk_projv(tb):
                    return lambda: proj_v(tb, load_x_tb("v", tb))

                def mk_drain(tb, eng=None):
                    return lambda: drain_norms(tb, eng)

                def mk_out(tb):
                    return lambda: out_proj(range(tb * 4, (tb + 1) * 4))

                for tb in range(NSTRIP):
                    fill = []
                    if tb + 1 < NSTRIP:
                        fill.append(mk_proj("q", tb + 1))
                    if tb == 0 and prev_tail is not None:
                        fill.append(prev_tail[0])   # prev body's drain
                    if tb >= 1:
                        fill.append(mk_drain(tb - 1))
                    if tb + 1 < NSTRIP:
                        fill += [mk_proj("k", tb + 1), mk_projv(tb + 1)]
                    if tb == 0 and prev_tail is not None:
                        fill.append(prev_tail[1])   # prev body's out_proj: late
                    if tb >= 2:
                        fill.append(mk_out(tb - 2))
                    flash_strip(tb, fill)
                out_proj(range((NSTRIP - 2) * 4, (NSTRIP - 1) * 4))
                return (mk_drain(NSTRIP - 1), mk_out(NSTRIP - 1))

            if reps > 1 and reps <= 4:
                # straight-line repetition (for TimelineSim marginal analysis)
                tail = None
                for _ in range(reps):
                    tail = body(tail)
                tail[0]()
                tail[1]()
            elif reps > 1:
                unroll = 4 if reps % 4 == 0 else 1
                with tc.For_i(0, reps // unroll, 1,
                              staggered_reset=True,
                              hint_engines=(mybir.EngineType.PE,
                                            mybir.EngineType.Activation,
                                            mybir.EngineType.DVE,
                                            mybir.EngineType.SP,
                                            mybir.EngineType.Pool)):
                    tail = None
                    for _ in range(unroll):
                        tail = body(tail)
                    tail[0]()
                    tail[1]()
            else:
                t0_, t1_ = body()
                t0_()
                t1_()

    nc.compile()
    return nc


_PROGRAM_CACHE = {}


def get_program(causal: bool, use_mask: bool, has_bias: bool, reps: int = 1):
    key = (causal, use_mask, has_bias, reps)
    if key not in _PROGRAM_CACHE:
        _PROGRAM_CACHE[key] = _build_program(causal, use_mask, has_bias, reps)
    return _PROGRAM_CACHE[key]


def _prep_in_maps(query, key, value, key_padding_mask, attn_mask,
                  Wq, bq, Wk, bk, Wv, bv, Wo, bo, use_mask, has_bias):
    """Build the 8 per-core input dicts."""
    import ml_dtypes
    cq, sq, ck, sk = _xpos_tables()
    tri = np.where(np.arange(128)[None, :] >= np.arange(128)[:, None],
                   np.float16(1.0), np.float16(0.0)).astype(np.float16)
    # rotate-half permutation: out row r = in row r^32 (symmetric involution)
    perm = np.zeros((128, 128), np.float16)
    perm[np.arange(128) ^ 32, np.arange(128)] = np.float16(1.0)

    def aug_x(x):
        a = np.empty((513, L), np.float16)
        a[0:512] = np.asarray(x, np.float32).T.astype(np.float16)
        a[512] = np.float16(1.0)
        return a

    xqTs = [aug_x(query[b]) for b in range(B)]
    xkTs = [aug_x(key[b]) for b in range(B)]
    xvTs = [aug_x(value[b]) for b in range(B)]

    masks = None
    if use_mask:
        am = np.asarray(attn_mask, np.float32)
        kp = np.asarray(key_padding_mask)
        masks = []
        for b in range(B):
            m = am.copy()
            if kp[b].any():
                m = m + np.where(kp[b], np.float32(-1e30),
                                 np.float32(0.0))[None, :]
            masks.append(np.ascontiguousarray(m.T.astype(np.float32)))

    Wq = np.asarray(Wq, np.float32); bq = np.asarray(bq, np.float32)
    Wk = np.asarray(Wk, np.float32); bk = np.asarray(bk, np.float32)
    Wv = np.asarray(Wv, np.float32); bv = np.asarray(bv, np.float32)
    Wo = np.asarray(Wo, np.float32)

    in_maps = []
    for core in range(N_CORES):
        b, hg = core // 2, core % 2
        hs = hg * HPC
        idx_p = np.concatenate(
            [hs * HD + hl * HD + _PERM64 for hl in range(HPC)])
        # sin-projection rows: within each head's 64-block, row r <- r XOR 32
        xor = (np.arange(256).reshape(HPC, HD)[:, (np.arange(HD) ^ 32)]
               ).reshape(-1)
        idx_s = idx_p[xor]
        idx_v = hs * HD + np.arange(HPC * HD)

        def aug_w(W, bias, idx):
            a = np.empty((513, 256), np.float32)
            a[0:512] = np.ascontiguousarray(W[idx, :].T)
            a[512] = bias[idx]
            return a.astype(ml_dtypes.bfloat16)

        m = {
            "xqT": xqTs[b], "xkT": xkTs[b], "xvT": xvTs[b],
            "wqcT": aug_w(Wq, bq, idx_p),
            "wkcT": aug_w(Wk, bk, idx_p),
            "wvT": aug_w(Wv, bv, idx_v),
            "woT": np.ascontiguousarray(Wo[:, idx_v].T).astype(ml_dtypes.bfloat16),
            "cq": cq, "sq": sq, "ck": ck, "sk": sk,
            "tri": tri, "perm": perm,
        }
        if use_mask:
            m["maskT"] = masks[b]
        in_maps.append(m)
    return in_maps


def classify_mask(attn_mask, key_padding_mask):
    am = np.asarray(attn_mask, np.float32)
    kp = np.asarray(key_padding_mask)
    if not kp.any():
        causal = np.where(
            np.tril(np.ones((L, L), bool)), np.float32(0.0),
            np.float32(NEG)).astype(np.float32)
        if np.array_equal(am, causal):
            return True, False
        if not am.any():
            return False, False
    return False, True


def kernel(query, key, value, key_padding_mask, attn_mask,
           Wq, bq, Wk, bk, Wv, bv, Wo, bo):
    causal, use_mask = classify_mask(attn_mask, key_padding_mask)
    has_bias = bool(np.asarray(bq).any() or np.asarray(bk).any()
                    or np.asarray(bv).any())
    nc = get_program(causal, use_mask, has_bias, reps=1)
    in_maps = _prep_in_maps(query, key, value, key_padding_mask, attn_mask,
                            Wq, bq, Wk, bk, Wv, bv, Wo, bo, use_mask, has_bias)
    res = run_bass_kernel_spmd(nc, in_maps, list(range(N_CORES)))
    bo = np.asarray(bo, np.float32)
    out = np.empty((B, L, EMBED), np.float32)
    for b in range(B):
        out[b] = (res.results[2 * b]["outp"].astype(np.float32)
                  + res.results[2 * b + 1]["outp"].astype(np.float32)
                  + bo[None, :])
    return out
